# revision 25
# baseline (speedup 1.0000x reference)
"""Trainium2 Bass kernel for nn_Encoder_Head_77343771066713 (DGCNN+PCT encoder).

Data-parallel over batch B=8 across 8 NeuronCores (one point cloud per core).
Self-contained: hardcodes all shapes. kernel(**inputs) -> (8, 256, 2048) f32.

v2 design (vs baseline):
  - neighbor-feature gathers use DMA-engine token gathers (dma_gather with
    16-bit transpose from an SBUF token buffer) instead of gpsimd ap_gather
  - conv layers: A-part (neighbor) conv is pre-applied before the gather
    (tokens = Wa@x per point, fp16); Q-part added post-gather; conv2 via
    column-tiled matmuls packs two row tiles into one PSUM tile; the max
    over k runs on raw conv2 PSUM (valid since BN scale > 0) and BN+leaky
    is applied once on the reduced [128, 1024] stack
  - kNN: -xx[n] row folded into the distance matmul (extra contraction row),
    f32r matmuls, 11-bit index encode into score mantissa (chunk=128 max8,
    no max_index), leaner phase 2
  - SA: replicated XQ enables 32-row-tiled energy matmuls, f32r colsum/
    broadcast matmuls, batched tails, grouped reciprocal
"""
import numpy as np

N = 2048
K = 40
B = 8
NCORES = 8
NT = N // 128          # 16 row tiles
EPS_BN = 1e-5

_COMPILED = None


def _build_program():
    import concourse.bass as bass
    import concourse.tile as tile
    from concourse import bacc, mybir

    f32 = mybir.dt.float32
    f32R = mybir.dt.float32r
    f16 = mybir.dt.float16
    u32 = mybir.dt.uint32
    u16 = mybir.dt.uint16
    i16 = mybir.dt.int16
    AF = mybir.ActivationFunctionType
    OP = mybir.AluOpType
    AX = mybir.AxisListType
    ts = bass.ts

    nc = bacc.Bacc("TRN2", target_bir_lowering=False, debug=False)

    def din(name, shape, dt=f32):
        return nc.dram_tensor(name, shape, dt, kind="ExternalInput")

    xt16 = din("xt16", [16, N])
    colenc = din("colenc", [128, N], u32)
    cst = din("cst", [128, 8], u32)
    ident64 = din("ident64", [64, 64])
    w1a_T = din("w1a_T", [4, 64])
    w1q_T = din("w1q_T", [4, 64])
    w2_Th = din("w2_Th", [64, 64], f16)
    bn2_s = din("bn2_s", [128, 1])
    bn2_b = din("bn2_b", [128, 1])
    w3a_T = din("w3a_T", [65, 64])
    w3q_T = din("w3q_T", [65, 64])
    w4_Th = din("w4_Th", [64, 64], f16)
    bn4_s = din("bn4_s", [128, 1])
    bn4_b = din("bn4_b", [128, 1])
    qk4 = din("qk4", [4, 128, 128])
    v_Tr = din("v_Tr", [4, 128, 128])
    vb_rep = din("vb_rep", [4, 128, 128])
    t_T = din("t_T", [4, 128, 128])
    tb_f = din("tb_f", [4, 128, 1])
    k1pat = din("k1pat", [4, 20])
    k1o = din("k1o", [1, 10])
    k2pat = din("k2pat", [64, 264])
    k2o = din("k2o", [1, 132])
    fuse_T = din("fuse_T", [512, 256])
    fuse_bc = din("fuse_bc", [128, 2])
    y_out = nc.dram_tensor("y", [256, N], f32, kind="ExternalOutput")

    with tile.TileContext(nc) as tc:
      with tc.tile_pool(name="consts", bufs=1) as consts, \
           tc.tile_pool(name="big", bufs=1) as big:
        xt_s = consts.tile([16, N], f32)
        nc.sync.dma_start(xt_s[:], xt16[:])
        colenc_s = consts.tile([128, N], u32)
        nc.sync.dma_start(colenc_s[:], colenc[:])
        cst_s = consts.tile([128, 8], u32)
        nc.sync.dma_start(cst_s[:], cst[:])
        MASKC = cst_s[:, 0:1]     # 0xFFFFFFC0
        C63 = cst_s[:, 1:2]       # 63
        CFFF8 = cst_s[:, 2:3]     # 0xFFF8
        C8 = cst_s[:, 3:4]        # 8
        id64 = consts.tile([64, 64], f32)
        nc.sync.dma_start(id64[:], ident64[:])

        w1a_s = consts.tile([4, 64], f32)
        nc.sync.dma_start(w1a_s[:], w1a_T[:])
        w1q_s = consts.tile([4, 64], f32)
        nc.sync.dma_start(w1q_s[:], w1q_T[:])
        w2_s = consts.tile([64, 64], f16)
        nc.sync.dma_start(w2_s[:], w2_Th[:])
        bn2s_s = consts.tile([128, 1], f32)
        nc.sync.dma_start(bn2s_s[:], bn2_s[:])
        bn2b_s = consts.tile([128, 1], f32)
        nc.sync.dma_start(bn2b_s[:], bn2_b[:])
        w3a_s = consts.tile([65, 64], f32)
        nc.sync.dma_start(w3a_s[:], w3a_T[:])
        w3q_s = consts.tile([65, 64], f32)
        nc.sync.dma_start(w3q_s[:], w3q_T[:])
        w4_s = consts.tile([64, 64], f16)
        nc.sync.dma_start(w4_s[:], w4_Th[:])
        bn4s_s = consts.tile([128, 1], f32)
        nc.sync.dma_start(bn4s_s[:], bn4_s[:])
        bn4b_s = consts.tile([128, 1], f32)
        nc.sync.dma_start(bn4b_s[:], bn4_b[:])
        fuse_s = consts.tile([128, 1024], f32)
        for k in range(4):
            nc.sync.dma_start(fuse_s[:, 256 * k:256 * k + 256],
                              fuse_T[128 * k:128 * k + 128, :])
        fuse_b_s = consts.tile([128, 2], f32)
        nc.sync.dma_start(fuse_b_s[:], fuse_bc[:])
        ones4 = consts.tile([4, 1], f32)
        nc.vector.memset(ones4[:], 1.0)
        ones64 = consts.tile([64, 1], f32)
        nc.vector.memset(ones64[:], 1.0)
        ones1 = consts.tile([128, 128], f32)
        nc.vector.memset(ones1[:], 1.0)
        ones_row = consts.tile([1, N], f32)
        nc.vector.memset(ones_row[:], 1.0)
        k1p_s = consts.tile([4, 20], f32)
        nc.sync.dma_start(k1p_s[:], k1pat[:])
        k1o_s = consts.tile([1, 10], f32)
        nc.sync.dma_start(k1o_s[:], k1o[:])
        k2p_s = consts.tile([64, 264], f32)
        nc.sync.dma_start(k2p_s[:], k2pat[:])
        k2o_s = consts.tile([1, 132], f32)
        nc.sync.dma_start(k2o_s[:], k2o[:])

        # ---- long-lived tensors ----
        tokd = big.tile([2048, 128], f16, tag="tokd", space="DRAM")
        tok = big.tile([128, N], f16, tag="tok")        # token buffer (reused)
        QQb = big.tile([64, N], f16, tag="QQ")          # Q-part (reused)
        idxw = big.tile([128, 5 * 1024], i16, tag="idxw")
        idxq = big.tile([16, 5 * 1024], i16, tag="idxq")
        stg_all = big.tile([128, 40 * NT], u16, tag="stg")
        x_stack = big.tile([128, 1024], f32, tag="xst")
        xsf = big.tile([128, 1024], f32, tag="xsf")
        hs = [big.tile([128, N], f32, name=f"h{i}", tag=f"h{i}")
              for i in range(5)]
        h0 = hs[0]

        # ================= token build =================
        # tok[p, 128r : 128r+64] = fp16(PP[:, 128r + p]); rest zero.
        def build_tokens(lhsA, lhsQ, rhs_ap, nK):
            nc.vector.memset(tok[:], 0.0)
            with tc.tile_pool(name="tbp", bufs=1, space="PSUM") as tbp, \
                 tc.tile_pool(name="tbs", bufs=1) as tbs, \
                 tc.tile_pool(name="trp", bufs=2, space="PSUM") as trp:
                pq_ps = tbp.tile([64, N], f32, tag="pq")
                for c in range(4):
                    nc.tensor.matmul(pq_ps[:, ts(c, 512)],
                                     lhsQ,
                                     rhs_ap[:, ts(c, 512)],
                                     start=True, stop=True)
                nc.scalar.activation(QQb[:], pq_ps[:], AF.Copy)
                for c in range(4):
                    nc.tensor.matmul(pq_ps[:, ts(c, 512)],
                                     lhsA,
                                     rhs_ap[:, ts(c, 512)],
                                     start=True, stop=True)
                pp_sb = tbs.tile([64, N], f32, tag="ppsb")
                nc.scalar.activation(pp_sb[:], pq_ps[:], AF.Copy)
                for r in range(NT):
                    tr = trp.tile([128, 64], f32, tag="tr")
                    nc.tensor.transpose(tr[:], pp_sb[:, ts(r, 128)], id64[:])
                    nc.scalar.activation(tok[:, 128 * r:128 * r + 64], tr[:],
                                         AF.Copy)
                for r in range(NT):
                    nc.sync.dma_start(tokd[128 * r:128 * r + 128, :],
                                      tok[:, 128 * r:128 * r + 128])

        # ================= kNN top-40 =================
        # A/B rows give negdist[n,m] = -xx[n] + 2<x_n,x_m> - xx[m] directly.
        def knn_topk(Amat, Bmat, nK):
            with tc.tile_pool(name="spp", bufs=2, space="PSUM") as spp, \
                 tc.tile_pool(name="scs", bufs=2) as scs, \
                 tc.tile_pool(name="sv2", bufs=2) as sv2:
                for t in range(NT):
                    spt = spp.tile([128, N], f32, tag="spt")
                    lhsT = Amat[:, t * 128:(t + 1) * 128]
                    for c in range(4):
                        nc.tensor.matmul(spt[:, ts(c, 512)],
                                         lhsT,
                                         Bmat[:, ts(c, 512)],
                                         start=True, stop=True)
                    scp = scs.tile([128, N], f32, tag="scp")
                    nc.scalar.activation(scp[:], spt[:], AF.Copy)
                    senc = scs.tile([128, N], f32, tag="senc")
                    colv = colenc_s[:, 0:128].rearrange(
                        "p (u c) -> p u c", u=1).broadcast_to((128, 16, 128))
                    nc.vector.scalar_tensor_tensor(
                        out=senc[:].bitcast(u32)
                        .rearrange("p (r c) -> p r c", r=16),
                        in0=scp[:].bitcast(u32)
                        .rearrange("p (r c) -> p r c", r=16),
                        scalar=MASKC, in1=colv,
                        op0=OP.bitwise_and, op1=OP.bitwise_or)
                    sv = sv2.tile([128, 256], f32, tag="sv")
                    for ch in range(32):
                        nc.vector.max(sv[:, 8 * ch:8 * ch + 8],
                                      senc[:, 64 * ch:64 * ch + 64])
                    m40 = sv2.tile([128, 40], f32, tag="m40")
                    pos = sv2.tile([128, 40], u32, tag="pos")
                    wk0 = sv2.tile([128, 256], f32, tag="wk0")
                    wk1 = sv2.tile([128, 256], f32, tag="wk1")
                    cur = sv
                    for r in range(5):
                        nc.vector.max(m40[:, 8 * r:8 * r + 8], cur[:])
                        nc.vector.max_index(pos[:, 8 * r:8 * r + 8],
                                            m40[:, 8 * r:8 * r + 8], cur[:])
                        if r < 4:
                            nxt = wk0 if r % 2 == 0 else wk1
                            nc.vector.match_replace(nxt[:],
                                                    m40[:, 8 * r:8 * r + 8],
                                                    cur[:], -3.0e38)
                            cur = nxt
                    loc = sv2.tile([128, 40], u32, tag="loc")
                    nc.vector.tensor_tensor(out=loc[:],
                                            in0=m40[:].bitcast(u32),
                                            in1=C63.broadcast_to((128, 40)),
                                            op=OP.bitwise_and)
                    chb = sv2.tile([128, 40], u32, tag="chb")
                    nc.vector.tensor_tensor(out=chb[:], in0=pos[:],
                                            in1=CFFF8.broadcast_to((128, 40)),
                                            op=OP.bitwise_and)
                    gid = sv2.tile([128, 40], u32, tag="gid")
                    nc.vector.tensor_tensor(out=gid[:], in0=chb[:],
                                            in1=C8.broadcast_to((128, 40)),
                                            op=OP.mult)
                    nc.vector.tensor_tensor(out=gid[:], in0=gid[:],
                                            in1=loc[:], op=OP.add)
                    nc.vector.tensor_copy(stg_all[:, 40 * t:40 * t + 40],
                                          gid[:])
            # stage indices: idxq[plo, 320t+40phi+j] = stg_all[16phi+plo, 40t+j]
            for phi in range(8):
                nc.sync.dma_start(
                    idxq[:].rearrange("p (t c) -> p t c", t=NT)
                        [:, :, 40 * phi:40 * phi + 40].bitcast(i16),
                    stg_all[16 * phi:16 * phi + 16, :]
                        .rearrange("p (t j) -> p t j", t=NT).bitcast(i16))
            for g in range(8):
                nc.sync.dma_start(idxw[16 * g:16 * g + 16, :], idxq[:])

        # ================= conv layer =================
        # per row tile t: gather 5120 tokens (idx i -> point 16*phi+plo,
        # nbr j with i = 640*phi + 16*j + plo), add Q, leaky, conv2 into
        # packed PSUM (two tiles -> 128 rows), max over j on raw PSUM.
        def conv_layer(w2T, bnS, bnB, hrow):
            with tc.tile_pool(name="gp", bufs=2) as gp, \
                 tc.tile_pool(name="lp", bufs=2) as lp, \
                 tc.tile_pool(name="cp", bufs=2, space="PSUM") as cp:
                for u in range(8):
                    lr = []
                    for half in range(2):
                        t = 2 * u + half
                        g = gp.tile([128, 5120], f16, tag="g")
                        nc.gpsimd.dma_gather(
                            out_ap=g[:].rearrange("p (u c) -> p u c", u=1),
                            in_ap=tokd[:],
                            idxs_ap=idxw[:, 320 * t:320 * t + 320],
                            num_idxs=5120,
                            num_idxs_reg=5120,
                            elem_size=128,
                            transpose=True,
                            queue_num=0,
                            single_packet=False)
                        summ = lp.tile([64, 5120], f16, tag="summ")
                        gv = g[0:64, :].rearrange(
                            "c (phi j plo) -> c phi j plo", phi=8, j=40)
                        qv = QQb[:, ts(t, 128)].rearrange(
                            "c (phi uu plo) -> c phi uu plo", uu=1, plo=16) \
                            .broadcast_to((64, 8, 40, 16))
                        sview = summ[:].rearrange(
                            "c (phi j plo) -> c phi j plo", phi=8, j=40)
                        nc.vector.tensor_tensor(out=sview, in0=gv, in1=qv,
                                                op=OP.add)
                        lr1 = lp.tile([64, 5120], f16, tag="lr1")
                        if half == 0:
                            nc.scalar.activation(lr1[:], summ[:], AF.Prelu,
                                                 alpha=0.2)
                        else:
                            nc.vector.scalar_tensor_tensor(
                                out=lr1[:], in0=summ[:], scalar=0.2,
                                in1=summ[:], op0=OP.mult, op1=OP.max)
                        lr.append(lr1)
                    for m in range(4):
                        ps = cp.tile([128, 1536], f32, tag="ps")
                        for half in range(2):
                            rw = 64 * half
                            for (o, w) in ((0, 512), (512, 512), (1024, 256)):
                                nc.tensor.matmul(
                                    ps[rw:rw + 64, o:o + w],
                                    w2T[:],
                                    lr[half][:, 1280 * m + o:1280 * m + o + w],
                                    start=True, stop=True)
                        nc.vector.tensor_reduce(
                            out=x_stack[:, 128 * u + 32 * m:
                                        128 * u + 32 * m + 32],
                            in_=ps[:, 0:1280].rearrange(
                                "c (phi j plo) -> c phi plo j", phi=2, j=40),
                            axis=AX.X, op=OP.max)
            # x = lrelu(bn_s * max + bn_b); write into h0 rows [hrow, hrow+64)
            nc.scalar.activation(xsf[:], x_stack[:], AF.Prelu,
                                 bias=bnB, scale=bnS, alpha=0.2)
            for u in range(8):
                nc.sync.dma_start(h0[hrow:hrow + 64, 256 * u:256 * u + 128],
                                  xsf[0:64, 128 * u:128 * u + 128])
                nc.sync.dma_start(h0[hrow:hrow + 64,
                                     256 * u + 128:256 * u + 256],
                                  xsf[64:128, 128 * u:128 * u + 128])

        # ================= layer 1 =================
        aug4 = big.tile([4, N], f32, tag="aug4")
        nc.scalar.activation(aug4[:], xt_s[0:4, :], AF.Copy)
        nc.vector.memset(aug4[0:1, :], 1.0)
        build_tokens(w1a_s[:], w1q_s[:], aug4[:], 4)

        with tc.tile_pool(name="kn1", bufs=1) as kn1:
            A1 = kn1.tile([5, N], f32, tag="A1")
            B1 = kn1.tile([5, N], f32, tag="B1")
            sq = kn1.tile([4, N], f32, tag="sq1")
            nc.scalar.activation(sq[:], xt_s[0:4, :], AF.Square)
            with tc.tile_pool(name="xxp", bufs=1, space="PSUM") as xxp:
                psA = xxp.tile([5, N], f32, tag="psA")
                psB = xxp.tile([5, N], f32, tag="psB")
                for c in range(4):
                    nc.tensor.matmul(psA[:, ts(c, 512)], k1p_s[:, 0:5],
                                     sq[:, ts(c, 512)], start=True, stop=False)
                    nc.tensor.matmul(psA[:, ts(c, 512)], k1p_s[:, 5:10],
                                     xt_s[0:4, ts(c, 512)],
                                     start=False, stop=False)
                    nc.tensor.matmul(psA[:, ts(c, 512)], k1o_s[:, 0:5],
                                     ones_row[:, ts(c, 512)],
                                     start=False, stop=True)
                    nc.tensor.matmul(psB[:, ts(c, 512)], k1p_s[:, 10:15],
                                     sq[:, ts(c, 512)], start=True, stop=False)
                    nc.tensor.matmul(psB[:, ts(c, 512)], k1p_s[:, 15:20],
                                     xt_s[0:4, ts(c, 512)],
                                     start=False, stop=False)
                    nc.tensor.matmul(psB[:, ts(c, 512)], k1o_s[:, 5:10],
                                     ones_row[:, ts(c, 512)],
                                     start=False, stop=True)
                nc.scalar.activation(A1[:], psA[:], AF.Copy)
                nc.scalar.activation(B1[:], psB[:], AF.Copy)
            knn_topk(A1[:], B1[:], 5)
        conv_layer(w2_s[:], bn2s_s[:], bn2b_s[:], 0)

        # ================= layer 2 =================
        with tc.tile_pool(name="ly2", bufs=1) as ly2:
            x1aug = ly2.tile([66, N], f32, tag="x1aug")
            nc.sync.dma_start(x1aug[0:64, :], h0[0:64, :])
            nc.vector.memset(x1aug[64:65, :], 1.0)
            build_tokens(w3a_s[:], w3q_s[:], x1aug[0:65, :], 65)

            A2 = ly2.tile([66, N], f32, tag="A2")
            B2 = ly2.tile([66, N], f32, tag="B2")
            sq2 = ly2.tile([64, N], f32, tag="sq2")
            nc.scalar.activation(sq2[:], x1aug[0:64, :], AF.Square)
            with tc.tile_pool(name="xx2p", bufs=1, space="PSUM") as xx2p:
                psA2 = xx2p.tile([66, N], f32, tag="psA2")
                for c in range(4):
                    nc.tensor.matmul(psA2[:, ts(c, 512)], k2p_s[:, 0:66],
                                     sq2[:, ts(c, 512)], start=True,
                                     stop=False)
                    nc.tensor.matmul(psA2[:, ts(c, 512)], k2p_s[:, 66:132],
                                     x1aug[0:64, ts(c, 512)],
                                     start=False, stop=False)
                    nc.tensor.matmul(psA2[:, ts(c, 512)], k2o_s[:, 0:66],
                                     ones_row[:, ts(c, 512)],
                                     start=False, stop=True)
                nc.scalar.activation(A2[:], psA2[:], AF.Copy)
            with tc.tile_pool(name="xx3p", bufs=1, space="PSUM") as xx3p:
                psB2 = xx3p.tile([66, N], f32, tag="psB2")
                for c in range(4):
                    nc.tensor.matmul(psB2[:, ts(c, 512)], k2p_s[:, 132:198],
                                     sq2[:, ts(c, 512)], start=True,
                                     stop=False)
                    nc.tensor.matmul(psB2[:, ts(c, 512)], k2p_s[:, 198:264],
                                     x1aug[0:64, ts(c, 512)],
                                     start=False, stop=False)
                    nc.tensor.matmul(psB2[:, ts(c, 512)], k2o_s[:, 66:132],
                                     ones_row[:, ts(c, 512)],
                                     start=False, stop=True)
                nc.scalar.activation(B2[:], psB2[:], AF.Copy)
            knn_topk(A2[:], B2[:], 66)
        conv_layer(w4_s[:], bn4s_s[:], bn4b_s[:], 64)

        # ================= SA layers =================
        with tc.tile_pool(name="saw", bufs=1) as saw, \
             tc.tile_pool(name="sal", bufs=1) as sal, \
             tc.tile_pool(name="sas", bufs=2) as sas:
            for l in range(4):
                h = hs[l]
                hn = hs[l + 1]
                qk_s = saw.tile([128, 128], f32, tag="qk")
                nc.sync.dma_start(qk_s[:], qk4[l])
                v_s = saw.tile([128, 128], f32, tag="vs")
                nc.sync.dma_start(v_s[:], v_Tr[l])
                vb_s = saw.tile([128, 128], f32, tag="vb")
                nc.sync.dma_start(vb_s[:], vb_rep[l])
                tT_s = saw.tile([128, 128], f32, tag="tT")
                nc.sync.dma_start(tT_s[:], t_T[l])
                tb_s = saw.tile([128, 1], f32, tag="tb")
                nc.sync.dma_start(tb_s[:], tb_f[l])
                XQ = saw.tile([128, N], f32R, tag="XQ")
                xvT = saw.tile([128, N], f32, tag="xvT")

                with tc.tile_pool(name=f"sap{l}", bufs=1, space="PSUM") as sap:
                    xqp = sap.tile([128, N], f32, tag="xq")
                    for c in range(4):
                        nc.tensor.matmul(xqp[:, ts(c, 512)],
                                         qk_s[:],
                                         h[:, ts(c, 512)],
                                         start=True, stop=True)
                    nc.scalar.activation(XQ[:], xqp[:], AF.Copy)
                with tc.tile_pool(name=f"sav{l}", bufs=1,
                                  space="PSUM") as sav:
                    xvp = sav.tile([128, N], f32, tag="xv")
                    for t in range(NT):
                        nc.tensor.matmul(xvp[:, ts(t, 128)],
                                         h[:, ts(t, 128)],
                                         v_s[:],
                                         start=True, stop=True)
                    nc.vector.tensor_tensor(
                        out=xvT[:].rearrange("p (r c) -> p r c", r=16),
                        in0=xvp[:].rearrange("p (r c) -> p r c", r=16),
                        in1=vb_s[:].rearrange("p (u c) -> p u c", u=1)
                        .broadcast_to((128, 16, 128)),
                        op=OP.add)

                with tc.tile_pool(name=f"sax{l}", bufs=1,
                                  space="PSUM") as sax:
                    xrp = sax.tile([128, N], f32, tag="xr")
                    rcs = sal.tile([64, 1024], f32, tag="rcs")
                    with tc.tile_pool(name=f"sac{l}", bufs=1,
                                      space="PSUM") as sac:
                        csA = sac.tile([64, 512], f32, tag="csA")
                        csB = sac.tile([64, 512], f32, tag="csB")
                        cst4 = [csA[0:1, :], csA[32:33, :],
                                csB[0:1, :], csB[32:33, :]]
                        with tc.tile_pool(name=f"sae{l}", bufs=2,
                                          space="PSUM") as sae:
                            for t in range(NT):
                                ext = sas.tile([128, N], f32R, tag="ex")
                                rs4 = sas.tile([128, 4], f32, tag="rs4")
                                for c in range(4):
                                    q = 32 * (c % 2)
                                    ep = sae.tile([128, 512], f32, tag="ep")
                                    nc.tensor.matmul(
                                        ep[:],
                                        XQ[q:q + 32, ts(t, 128)],
                                        XQ[q:q + 32, ts(c, 512)],
                                        start=True, stop=True)
                                    nc.scalar.activation(
                                        ext[:, ts(c, 512)], ep[:], AF.Exp,
                                        accum_out=rs4[:, c:c + 1])
                                rsum = sas.tile([128, 1], f32, tag="rsum")
                                nc.vector.tensor_reduce(out=rsum[:],
                                                        in_=rs4[:],
                                                        axis=AX.X, op=OP.add)
                                invr = sas.tile([128, 1], f32, tag="invr")
                                nc.vector.reciprocal(invr[:], rsum[:])

                                xvl = sas.tile([128, 128], f32R, tag="xvl")
                                nc.scalar.activation(xvl[:],
                                                     xvT[:, ts(t, 128)],
                                                     AF.Copy, scale=invr[:])
                                for c in range(4):
                                    nc.tensor.matmul(
                                        cst4[c], invr[:],
                                        ext[:, ts(c, 512)].bitcast(f32),
                                        start=(t == 0), stop=(t == NT - 1))
                                for c in range(4):
                                    nc.tensor.matmul(
                                        xrp[:, ts(c, 512)], xvl[:],
                                        ext[:, ts(c, 512)],
                                        start=(t == 0), stop=(t == NT - 1))
                        # 1/(colsum + eps): rows {0,32} x two col halves
                        cssb = sal.tile([64, 1024], f32, tag="cssb")
                        nc.scalar.activation(cssb[:, 0:512], csA[:], AF.Copy)
                        nc.scalar.activation(cssb[:, 512:1024], csB[:],
                                             AF.Copy)
                    # 1/(cs+eps) = exp(-ln(cs+eps)) on ACT (lane-legal)
                    for base in (0, 32):
                        nc.scalar.activation(rcs[base:base + 1, :],
                                             cssb[base:base + 1, :],
                                             AF.Ln)
                        nc.scalar.activation(rcs[base:base + 1, :],
                                             rcs[base:base + 1, :],
                                             AF.Exp, scale=-1.0)
                    rcs4 = [rcs[0:1, 0:512], rcs[32:33, 0:512],
                            rcs[0:1, 512:1024], rcs[32:33, 512:1024]]
                    d = sal.tile([128, N], f32, tag="d")
                    with tc.tile_pool(name=f"sab{l}", bufs=1,
                                      space="PSUM") as sab:
                        rbp = sab.tile([128, N], f32, tag="rb")
                        for c in range(4):
                            base = 32 * (c % 2)
                            nc.tensor.matmul(
                                rbp[:, ts(c, 512)],
                                ones1[base:base + 1, :],
                                rcs4[c],
                                start=True, stop=True)
                        rbs = sal.tile([128, N], f32, tag="rbs")
                        nc.scalar.activation(rbs[:], rbp[:], AF.Copy)
                        xrn = sal.tile([128, N], f32, tag="xrn")
                        nc.vector.tensor_tensor(out=xrn[:], in0=xrp[:],
                                                in1=rbs[:], op=OP.mult)
                        nc.vector.tensor_tensor(out=d[:], in0=h[:],
                                                in1=xrn[:], op=OP.subtract)
                with tc.tile_pool(name=f"sad{l}", bufs=2, space="PSUM") as sad:
                    for c in range(4):
                        drp = sad.tile([128, 512], f32, tag="dr")
                        nc.tensor.matmul(drp[:], tT_s[:],
                                         d[:, ts(c, 512)],
                                         start=True, stop=True)
                        rl = sas.tile([128, 512], f32, tag="rl")
                        nc.scalar.activation(rl[:], drp[:], AF.Relu,
                                             bias=tb_s[:], scale=1.0)
                        nc.vector.tensor_tensor(out=hn[:, ts(c, 512)],
                                                in0=h[:, ts(c, 512)],
                                                in1=rl[:], op=OP.add)

        # ================= fuse =================
        with tc.tile_pool(name="fup", bufs=2, space="PSUM") as fup, \
             tc.tile_pool(name="fus", bufs=2) as fus:
            for o in range(2):
                for c in range(4):
                    fp = fup.tile([128, 512], f32, tag="fp")
                    for k in range(4):
                        nc.tensor.matmul(
                            fp[:],
                            fuse_s[:, 256 * k + 128 * o:
                                   256 * k + 128 * o + 128],
                            hs[k + 1][:, ts(c, 512)],
                            start=(k == 0), stop=(k == 3))
                    yt = fus.tile([128, 512], f32, tag="yt")
                    nc.scalar.activation(yt[:], fp[:], AF.Prelu,
                                         bias=fuse_b_s[:, o:o + 1],
                                         scale=1.0, alpha=0.2)
                    nc.sync.dma_start(y_out[128 * o:128 * o + 128, ts(c, 512)],
                                      yt[:])

    nc.compile()
    return nc


def _prep_inputs(inputs):
    """Host-side: fold BN into weights, build per-core in_maps."""
    f = lambda a: np.ascontiguousarray(np.asarray(a, np.float32))
    x = f(inputs["x"])                      # (8, 2048, 3)
    scale = lambda g: (np.asarray(g, np.float32)
                       / np.sqrt(np.float32(1.0 + EPS_BN)))

    s1, b1 = scale(inputs["g1"]), f(inputs["b1"])
    w1 = f(inputs["w1"])
    w1a = s1[:, None] * w1[:, 0:3]                       # (64, 3)
    w1q = np.concatenate([b1[:, None],
                          s1[:, None] * (w1[:, 3:6] - w1[:, 0:3])],
                         axis=1)                         # (64, 4)
    w1a4 = np.zeros((4, 64), np.float32)
    w1a4[1:4, :] = w1a.T
    s2v, b2v = scale(inputs["g2"]), f(inputs["b2"])
    s3, b3 = scale(inputs["g3"]), f(inputs["b3"])
    w3 = f(inputs["w3"])
    w3a = np.concatenate([s3[:, None] * w3[:, 0:64],
                          np.zeros((64, 1), np.float32)], axis=1)  # (64, 65)
    w3q = np.concatenate([s3[:, None] * (w3[:, 64:128] - w3[:, 0:64]),
                          b3[:, None]], axis=1)          # (64, 65)
    s4v, b4v = scale(inputs["g4"]), f(inputs["b4"])

    qk = f(inputs["sa_qk"])                 # (4, 32, 128)
    v = f(inputs["sa_v"])                   # (4, 128, 128)
    vb = f(inputs["sa_vb"])                 # (4, 128)
    t = f(inputs["sa_t"])
    tb = f(inputs["sa_tb"])
    sg = scale(inputs["sa_g"])              # (4, 128)
    sb = f(inputs["sa_b"])
    fuse_w = f(inputs["fuse_w"])
    sf = scale(inputs["fuse_g"])
    fb = f(inputs["fuse_b"])                # (256,)

    qk_T = np.transpose(qk, (0, 2, 1))      # (4, 128, 32)

    # kNN A/B construction patterns:
    # A1 = [1s; 2x; -xx], B1 = [-xx; x; 1s] (contraction rows 0..4)
    k1As = np.zeros((4, 5), np.float32); k1As[:, 4] = -1.0
    k1Ax = np.zeros((4, 5), np.float32)
    for j in range(3): k1Ax[1 + j, 1 + j] = 2.0
    k1Bs = np.zeros((4, 5), np.float32); k1Bs[:, 0] = -1.0
    k1Bx = np.zeros((4, 5), np.float32)
    for j in range(3): k1Bx[1 + j, 1 + j] = 1.0
    k1o_h = np.zeros((1, 10), np.float32); k1o_h[0, 0] = 1.0; k1o_h[0, 9] = 1.0
    k2As = np.zeros((64, 66), np.float32); k2As[:, 65] = -1.0
    k2Ax = np.zeros((64, 66), np.float32); k2Ax[:, 0:64] = 2.0 * np.eye(64)
    k2Bs = np.zeros((64, 66), np.float32); k2Bs[:, 64] = -1.0
    k2Bx = np.zeros((64, 66), np.float32); k2Bx[:, 0:64] = np.eye(64)
    k2o_h = np.zeros((1, 132), np.float32)
    k2o_h[0, 64] = 1.0; k2o_h[0, 66 + 65] = 1.0

    common = {
        "k1pat": np.ascontiguousarray(
            np.concatenate([k1As, k1Ax, k1Bs, k1Bx], axis=1)),
        "k1o": k1o_h,
        "k2pat": np.ascontiguousarray(
            np.concatenate([k2As, k2Ax, k2Bs, k2Bx], axis=1)),
        "k2o": k2o_h,
        "colenc": np.tile(np.arange(64, dtype=np.uint32), N // 64)[None, :]
                    .repeat(128, 0).copy(),
        "cst": np.array([0xFFFFFFC0, 63, 0xFFF8, 8, 0, 0, 0, 0],
                        dtype=np.uint32)[None, :].repeat(128, 0).copy(),
        "ident64": np.eye(64, dtype=np.float32),
        "w1a_T": np.ascontiguousarray(w1a4),
        "w1q_T": np.ascontiguousarray(w1q.T),
        "w2_Th": np.ascontiguousarray(f(inputs["w2"]).T.astype(np.float16)),
        "bn2_s": np.tile(s2v, 2)[:, None].copy(),
        "bn2_b": np.tile(b2v, 2)[:, None].copy(),
        "w3a_T": np.ascontiguousarray(w3a.T),
        "w3q_T": np.ascontiguousarray(w3q.T),
        "w4_Th": np.ascontiguousarray(f(inputs["w4"]).T.astype(np.float16)),
        "bn4_s": np.tile(s4v, 2)[:, None].copy(),
        "bn4_b": np.tile(b4v, 2)[:, None].copy(),
        "qk4": np.ascontiguousarray(np.tile(qk_T, (1, 1, 4))),
        "v_Tr": np.ascontiguousarray(np.transpose(v, (0, 2, 1))),
        "vb_rep": np.ascontiguousarray(vb[:, None, :].repeat(128, 1)),
        "t_T": np.ascontiguousarray(
            np.transpose(sg[:, :, None] * t, (0, 2, 1))),
        "tb_f": np.ascontiguousarray((sg * tb + sb)[:, :, None]),
        "fuse_T": np.ascontiguousarray((sf[:, None] * fuse_w).T),
        "fuse_bc": np.ascontiguousarray(fb.reshape(2, 128).T),
    }
    in_maps = []
    for b in range(B):
        xt = np.zeros((16, N), np.float32)
        xt[1:4, :] = x[b].T
        m = dict(common)
        m["xt16"] = xt
        in_maps.append(m)
    return in_maps


def kernel(**inputs):
    global _COMPILED
    from concourse.bass_utils import run_bass_kernel_spmd
    if _COMPILED is None:
        _COMPILED = _build_program()
    nc = _COMPILED
    in_maps = _prep_inputs(inputs)
    res = run_bass_kernel_spmd(nc, in_maps, list(range(NCORES)))
    out = np.stack([res.results[b]["y"] for b in range(B)], axis=0)
    return out.astype(np.float32)


# revision 26
# speedup vs baseline: 1.0940x; 1.0940x over previous
"""Trainium2 Bass kernel for nn_Encoder_Head_77343771066713 (DGCNN+PCT encoder).

Data-parallel over batch B=8 across 8 NeuronCores (one point cloud per core).
Self-contained: hardcodes all shapes. kernel(**inputs) -> (8, 256, 2048) f32.

v2 design (vs baseline):
  - neighbor-feature gathers use DMA-engine token gathers (dma_gather with
    16-bit transpose from an SBUF token buffer) instead of gpsimd ap_gather
  - conv layers: A-part (neighbor) conv is pre-applied before the gather
    (tokens = Wa@x per point, fp16); Q-part added post-gather; conv2 via
    column-tiled matmuls packs two row tiles into one PSUM tile; the max
    over k runs on raw conv2 PSUM (valid since BN scale > 0) and BN+leaky
    is applied once on the reduced [128, 1024] stack
  - kNN: -xx[n] row folded into the distance matmul (extra contraction row),
    f32r matmuls, 11-bit index encode into score mantissa (chunk=128 max8,
    no max_index), leaner phase 2
  - SA: replicated XQ enables 32-row-tiled energy matmuls, f32r colsum/
    broadcast matmuls, batched tails, grouped reciprocal
"""
import numpy as np

N = 2048
K = 40
B = 8
NCORES = 8
NT = N // 128          # 16 row tiles
EPS_BN = 1e-5

_COMPILED = None


def _build_program():
    import concourse.bass as bass
    import concourse.tile as tile
    from concourse import bacc, mybir

    f32 = mybir.dt.float32
    f32R = mybir.dt.float32r
    f16 = mybir.dt.float16
    u32 = mybir.dt.uint32
    u16 = mybir.dt.uint16
    i16 = mybir.dt.int16
    AF = mybir.ActivationFunctionType
    OP = mybir.AluOpType
    AX = mybir.AxisListType
    ts = bass.ts

    nc = bacc.Bacc("TRN2", target_bir_lowering=False, debug=False)

    def din(name, shape, dt=f32):
        return nc.dram_tensor(name, shape, dt, kind="ExternalInput")

    xt16 = din("xt16", [16, N])
    colenc = din("colenc", [128, N], u32)
    cst = din("cst", [128, 8], u32)
    ident64 = din("ident64", [64, 64])
    w1a_T = din("w1a_T", [4, 64])
    w1q_T = din("w1q_T", [4, 64])
    w2_Th = din("w2_Th", [64, 64], f16)
    bn2_s = din("bn2_s", [128, 1])
    bn2_b = din("bn2_b", [128, 1])
    w3a_T = din("w3a_T", [65, 64])
    w3q_T = din("w3q_T", [65, 64])
    w4_Th = din("w4_Th", [64, 64], f16)
    bn4_s = din("bn4_s", [128, 1])
    bn4_b = din("bn4_b", [128, 1])
    qk4 = din("qk4", [4, 128, 128])
    v_Tr = din("v_Tr", [4, 128, 128])
    vb_rep = din("vb_rep", [4, 128, 128])
    t_T = din("t_T", [4, 128, 128])
    tb_f = din("tb_f", [4, 128, 1])
    k1pat = din("k1pat", [4, 20])
    k1o = din("k1o", [1, 10])
    k2pat = din("k2pat", [64, 264])
    k2o = din("k2o", [1, 132])
    fuse_T = din("fuse_T", [512, 256])
    fuse_bc = din("fuse_bc", [128, 2])
    y_out = nc.dram_tensor("y", [256, N], f32, kind="ExternalOutput")

    with tile.TileContext(nc) as tc:
      with tc.tile_pool(name="consts", bufs=1) as consts, \
           tc.tile_pool(name="big", bufs=1) as big:
        xt_s = consts.tile([16, N], f32)
        nc.sync.dma_start(xt_s[:], xt16[:])
        colenc_s = consts.tile([128, N], u32)
        nc.sync.dma_start(colenc_s[:], colenc[:])
        cst_s = consts.tile([128, 8], u32)
        nc.sync.dma_start(cst_s[:], cst[:])
        MASKC = cst_s[:, 0:1]     # 0xFFFFFFC0
        C63 = cst_s[:, 1:2]       # 63
        CFFF8 = cst_s[:, 2:3]     # 0xFFF8
        C8 = cst_s[:, 3:4]        # 8
        id64 = consts.tile([64, 64], f32)
        nc.sync.dma_start(id64[:], ident64[:])

        w1a_s = consts.tile([4, 64], f32)
        nc.sync.dma_start(w1a_s[:], w1a_T[:])
        w1q_s = consts.tile([4, 64], f32)
        nc.sync.dma_start(w1q_s[:], w1q_T[:])
        w2_s = consts.tile([64, 64], f16)
        nc.sync.dma_start(w2_s[:], w2_Th[:])
        bn2s_s = consts.tile([128, 1], f32)
        nc.sync.dma_start(bn2s_s[:], bn2_s[:])
        bn2b_s = consts.tile([128, 1], f32)
        nc.sync.dma_start(bn2b_s[:], bn2_b[:])
        w3a_s = consts.tile([65, 64], f32)
        nc.sync.dma_start(w3a_s[:], w3a_T[:])
        w3q_s = consts.tile([65, 64], f32)
        nc.sync.dma_start(w3q_s[:], w3q_T[:])
        w4_s = consts.tile([64, 64], f16)
        nc.sync.dma_start(w4_s[:], w4_Th[:])
        bn4s_s = consts.tile([128, 1], f32)
        nc.sync.dma_start(bn4s_s[:], bn4_s[:])
        bn4b_s = consts.tile([128, 1], f32)
        nc.sync.dma_start(bn4b_s[:], bn4_b[:])
        fuse_s = consts.tile([128, 1024], f32)
        for k in range(4):
            nc.sync.dma_start(fuse_s[:, 256 * k:256 * k + 256],
                              fuse_T[128 * k:128 * k + 128, :])
        fuse_b_s = consts.tile([128, 2], f32)
        nc.sync.dma_start(fuse_b_s[:], fuse_bc[:])
        ones4 = consts.tile([4, 1], f32)
        nc.vector.memset(ones4[:], 1.0)
        ones64 = consts.tile([64, 1], f32)
        nc.vector.memset(ones64[:], 1.0)
        ones1 = consts.tile([128, 128], f32)
        nc.vector.memset(ones1[:], 1.0)
        ones_row = consts.tile([1, N], f32)
        nc.vector.memset(ones_row[:], 1.0)
        k1p_s = consts.tile([4, 20], f32)
        nc.sync.dma_start(k1p_s[:], k1pat[:])
        k1o_s = consts.tile([1, 10], f32)
        nc.sync.dma_start(k1o_s[:], k1o[:])
        k2p_s = consts.tile([64, 264], f32)
        nc.sync.dma_start(k2p_s[:], k2pat[:])
        k2o_s = consts.tile([1, 132], f32)
        nc.sync.dma_start(k2o_s[:], k2o[:])

        # ---- long-lived tensors ----
        tokd = big.tile([2048, 128], f16, tag="tokd", space="DRAM")
        tok = big.tile([128, N], f16, tag="tok")        # token buffer (reused)
        QQb = big.tile([64, N], f16, tag="QQ")          # Q-part (reused)
        idxw = big.tile([128, 5 * 1024], i16, tag="idxw")
        idxq = big.tile([16, 5 * 1024], i16, tag="idxq")
        stg_all = big.tile([128, 40 * NT], u16, tag="stg")
        x_stack = big.tile([128, 1024], f32, tag="xst")
        xsf = big.tile([128, 1024], f32, tag="xsf")
        hs = [big.tile([128, N], f32, name=f"h{i}", tag=f"h{i}")
              for i in range(5)]
        h0 = hs[0]

        # ================= token build =================
        # tok[p, 128r : 128r+64] = fp16(PP[:, 128r + p]); rest zero.
        def build_tokens(lhsA, lhsQ, rhs_ap, nK):
            nc.vector.memset(tok[:], 0.0)
            with tc.tile_pool(name="tbp", bufs=1, space="PSUM") as tbp, \
                 tc.tile_pool(name="tbs", bufs=1) as tbs, \
                 tc.tile_pool(name="trp", bufs=2, space="PSUM") as trp:
                pq_ps = tbp.tile([64, N], f32, tag="pq")
                for c in range(4):
                    nc.tensor.matmul(pq_ps[:, ts(c, 512)],
                                     lhsQ,
                                     rhs_ap[:, ts(c, 512)],
                                     start=True, stop=True)
                nc.scalar.activation(QQb[:], pq_ps[:], AF.Copy)
                for c in range(4):
                    nc.tensor.matmul(pq_ps[:, ts(c, 512)],
                                     lhsA,
                                     rhs_ap[:, ts(c, 512)],
                                     start=True, stop=True)
                pp_sb = tbs.tile([64, N], f32, tag="ppsb")
                nc.scalar.activation(pp_sb[:], pq_ps[:], AF.Copy)
                for r in range(NT):
                    tr = trp.tile([128, 64], f32, tag="tr")
                    nc.tensor.transpose(tr[:], pp_sb[:, ts(r, 128)], id64[:])
                    nc.scalar.activation(tok[:, 128 * r:128 * r + 64], tr[:],
                                         AF.Copy)
                for r in range(NT):
                    nc.sync.dma_start(tokd[128 * r:128 * r + 128, :],
                                      tok[:, 128 * r:128 * r + 128])

        # ================= kNN top-40 =================
        # A/B rows give negdist[n,m] = -xx[n] + 2<x_n,x_m> - xx[m] directly.
        def knn_topk(Amat, Bmat, nK):
            with tc.tile_pool(name="spp", bufs=2, space="PSUM") as spp, \
                 tc.tile_pool(name="scs", bufs=2) as scs, \
                 tc.tile_pool(name="sv2", bufs=2) as sv2:
                for t in range(NT):
                    spt = spp.tile([128, N], f32, tag="spt")
                    lhsT = Amat[:, t * 128:(t + 1) * 128]
                    for c in range(4):
                        nc.tensor.matmul(spt[:, ts(c, 512)],
                                         lhsT,
                                         Bmat[:, ts(c, 512)],
                                         start=True, stop=True)
                    scp = scs.tile([128, N], f32, tag="scp")
                    nc.scalar.activation(scp[:], spt[:], AF.Copy)
                    senc = scs.tile([128, N], f32, tag="senc")
                    colv = colenc_s[:, 0:128].rearrange(
                        "p (u c) -> p u c", u=1).broadcast_to((128, 16, 128))
                    nc.vector.scalar_tensor_tensor(
                        out=senc[:].bitcast(u32)
                        .rearrange("p (r c) -> p r c", r=16),
                        in0=scp[:].bitcast(u32)
                        .rearrange("p (r c) -> p r c", r=16),
                        scalar=MASKC, in1=colv,
                        op0=OP.bitwise_and, op1=OP.bitwise_or)
                    sv = sv2.tile([128, 256], f32, tag="sv")
                    for ch in range(32):
                        nc.vector.max(sv[:, 8 * ch:8 * ch + 8],
                                      senc[:, 64 * ch:64 * ch + 64])
                    m40 = sv2.tile([128, 40], f32, tag="m40")
                    pos = sv2.tile([128, 40], u32, tag="pos")
                    wk0 = sv2.tile([128, 256], f32, tag="wk0")
                    wk1 = sv2.tile([128, 256], f32, tag="wk1")
                    cur = sv
                    for r in range(5):
                        nc.vector.max(m40[:, 8 * r:8 * r + 8], cur[:])
                        nc.vector.max_index(pos[:, 8 * r:8 * r + 8],
                                            m40[:, 8 * r:8 * r + 8], cur[:])
                        if r < 4:
                            nxt = wk0 if r % 2 == 0 else wk1
                            nc.vector.match_replace(nxt[:],
                                                    m40[:, 8 * r:8 * r + 8],
                                                    cur[:], -3.0e38)
                            cur = nxt
                    loc = sv2.tile([128, 40], u32, tag="loc")
                    nc.vector.tensor_tensor(out=loc[:],
                                            in0=m40[:].bitcast(u32),
                                            in1=C63.broadcast_to((128, 40)),
                                            op=OP.bitwise_and)
                    chb = sv2.tile([128, 40], u32, tag="chb")
                    nc.vector.tensor_tensor(out=chb[:], in0=pos[:],
                                            in1=CFFF8.broadcast_to((128, 40)),
                                            op=OP.bitwise_and)
                    gid = sv2.tile([128, 40], u32, tag="gid")
                    nc.vector.tensor_tensor(out=gid[:], in0=chb[:],
                                            in1=C8.broadcast_to((128, 40)),
                                            op=OP.mult)
                    nc.vector.tensor_tensor(out=gid[:], in0=gid[:],
                                            in1=loc[:], op=OP.add)
                    nc.vector.tensor_copy(stg_all[:, 40 * t:40 * t + 40],
                                          gid[:])
            # stage indices: idxq[plo, 320t+40phi+j] = stg_all[16phi+plo, 40t+j]
            for phi in range(8):
                nc.sync.dma_start(
                    idxq[:].rearrange("p (t c) -> p t c", t=NT)
                        [:, :, 40 * phi:40 * phi + 40].bitcast(i16),
                    stg_all[16 * phi:16 * phi + 16, :]
                        .rearrange("p (t j) -> p t j", t=NT).bitcast(i16))
            for g in range(8):
                nc.sync.dma_start(idxw[16 * g:16 * g + 16, :], idxq[:])

        # ================= conv layer =================
        # per row tile t: gather 5120 tokens (idx i -> point 16*phi+plo,
        # nbr j with i = 640*phi + 16*j + plo), add Q, leaky, conv2 into
        # packed PSUM (two tiles -> 128 rows), max over j on raw PSUM.
        def conv_layer(w2T, bnS, bnB, hrow):
            with tc.tile_pool(name="gp", bufs=2) as gp, \
                 tc.tile_pool(name="lp", bufs=2) as lp, \
                 tc.tile_pool(name="cp", bufs=2, space="PSUM") as cp:
                for u in range(8):
                    lr = []
                    for half in range(2):
                        t = 2 * u + half
                        g = gp.tile([128, 5120], f16, tag="g")
                        for gc in range(10):
                            nc.gpsimd.dma_gather(
                                out_ap=g[:, 512 * gc:512 * gc + 512]
                                    .rearrange("p (u c) -> p u c", u=1),
                                in_ap=tokd[:],
                                idxs_ap=idxw[:, 320 * t + 32 * gc:
                                             320 * t + 32 * gc + 32],
                                num_idxs=512,
                                num_idxs_reg=512,
                                elem_size=128,
                                transpose=True,
                                queue_num=0)
                        summ = lp.tile([64, 5120], f16, tag="summ")
                        gv = g[0:64, :].rearrange(
                            "c (phi j plo) -> c phi j plo", phi=8, j=40)
                        qv = QQb[:, ts(t, 128)].rearrange(
                            "c (phi uu plo) -> c phi uu plo", uu=1, plo=16) \
                            .broadcast_to((64, 8, 40, 16))
                        sview = summ[:].rearrange(
                            "c (phi j plo) -> c phi j plo", phi=8, j=40)
                        nc.vector.tensor_tensor(out=sview, in0=gv, in1=qv,
                                                op=OP.add)
                        lr1 = lp.tile([64, 5120], f16, tag="lr1")
                        if half == 0:
                            nc.scalar.activation(lr1[:], summ[:], AF.Prelu,
                                                 alpha=0.2)
                        else:
                            nc.vector.scalar_tensor_tensor(
                                out=lr1[:], in0=summ[:], scalar=0.2,
                                in1=summ[:], op0=OP.mult, op1=OP.max)
                        lr.append(lr1)
                    for m in range(4):
                        ps = cp.tile([128, 1536], f32, tag="ps")
                        for half in range(2):
                            rw = 64 * half
                            for (o, w) in ((0, 512), (512, 512), (1024, 256)):
                                nc.tensor.matmul(
                                    ps[rw:rw + 64, o:o + w],
                                    w2T[:],
                                    lr[half][:, 1280 * m + o:1280 * m + o + w],
                                    start=True, stop=True)
                        nc.vector.tensor_reduce(
                            out=x_stack[:, 128 * u + 32 * m:
                                        128 * u + 32 * m + 32],
                            in_=ps[:, 0:1280].rearrange(
                                "c (phi j plo) -> c phi plo j", phi=2, j=40),
                            axis=AX.X, op=OP.max)
            # x = lrelu(bn_s * max + bn_b); write into h0 rows [hrow, hrow+64)
            nc.scalar.activation(xsf[:], x_stack[:], AF.Prelu,
                                 bias=bnB, scale=bnS, alpha=0.2)
            for u in range(8):
                nc.sync.dma_start(h0[hrow:hrow + 64, 256 * u:256 * u + 128],
                                  xsf[0:64, 128 * u:128 * u + 128])
                nc.sync.dma_start(h0[hrow:hrow + 64,
                                     256 * u + 128:256 * u + 256],
                                  xsf[64:128, 128 * u:128 * u + 128])

        # ================= layer 1 =================
        aug4 = big.tile([4, N], f32, tag="aug4")
        nc.scalar.activation(aug4[:], xt_s[0:4, :], AF.Copy)
        nc.vector.memset(aug4[0:1, :], 1.0)
        build_tokens(w1a_s[:], w1q_s[:], aug4[:], 4)

        with tc.tile_pool(name="kn1", bufs=1) as kn1:
            A1 = kn1.tile([5, N], f32, tag="A1")
            B1 = kn1.tile([5, N], f32, tag="B1")
            sq = kn1.tile([4, N], f32, tag="sq1")
            nc.scalar.activation(sq[:], xt_s[0:4, :], AF.Square)
            with tc.tile_pool(name="xxp", bufs=1, space="PSUM") as xxp:
                psA = xxp.tile([5, N], f32, tag="psA")
                psB = xxp.tile([5, N], f32, tag="psB")
                for c in range(4):
                    nc.tensor.matmul(psA[:, ts(c, 512)], k1p_s[:, 0:5],
                                     sq[:, ts(c, 512)], start=True, stop=False)
                    nc.tensor.matmul(psA[:, ts(c, 512)], k1p_s[:, 5:10],
                                     xt_s[0:4, ts(c, 512)],
                                     start=False, stop=False)
                    nc.tensor.matmul(psA[:, ts(c, 512)], k1o_s[:, 0:5],
                                     ones_row[:, ts(c, 512)],
                                     start=False, stop=True)
                    nc.tensor.matmul(psB[:, ts(c, 512)], k1p_s[:, 10:15],
                                     sq[:, ts(c, 512)], start=True, stop=False)
                    nc.tensor.matmul(psB[:, ts(c, 512)], k1p_s[:, 15:20],
                                     xt_s[0:4, ts(c, 512)],
                                     start=False, stop=False)
                    nc.tensor.matmul(psB[:, ts(c, 512)], k1o_s[:, 5:10],
                                     ones_row[:, ts(c, 512)],
                                     start=False, stop=True)
                nc.scalar.activation(A1[:], psA[:], AF.Copy)
                nc.scalar.activation(B1[:], psB[:], AF.Copy)
            knn_topk(A1[:], B1[:], 5)
        conv_layer(w2_s[:], bn2s_s[:], bn2b_s[:], 0)

        # ================= layer 2 =================
        with tc.tile_pool(name="ly2", bufs=1) as ly2:
            x1aug = ly2.tile([66, N], f32, tag="x1aug")
            nc.sync.dma_start(x1aug[0:64, :], h0[0:64, :])
            nc.vector.memset(x1aug[64:65, :], 1.0)
            build_tokens(w3a_s[:], w3q_s[:], x1aug[0:65, :], 65)

            A2 = ly2.tile([66, N], f32, tag="A2")
            B2 = ly2.tile([66, N], f32, tag="B2")
            sq2 = ly2.tile([64, N], f32, tag="sq2")
            nc.scalar.activation(sq2[:], x1aug[0:64, :], AF.Square)
            with tc.tile_pool(name="xx2p", bufs=1, space="PSUM") as xx2p:
                psA2 = xx2p.tile([66, N], f32, tag="psA2")
                for c in range(4):
                    nc.tensor.matmul(psA2[:, ts(c, 512)], k2p_s[:, 0:66],
                                     sq2[:, ts(c, 512)], start=True,
                                     stop=False)
                    nc.tensor.matmul(psA2[:, ts(c, 512)], k2p_s[:, 66:132],
                                     x1aug[0:64, ts(c, 512)],
                                     start=False, stop=False)
                    nc.tensor.matmul(psA2[:, ts(c, 512)], k2o_s[:, 0:66],
                                     ones_row[:, ts(c, 512)],
                                     start=False, stop=True)
                nc.scalar.activation(A2[:], psA2[:], AF.Copy)
            with tc.tile_pool(name="xx3p", bufs=1, space="PSUM") as xx3p:
                psB2 = xx3p.tile([66, N], f32, tag="psB2")
                for c in range(4):
                    nc.tensor.matmul(psB2[:, ts(c, 512)], k2p_s[:, 132:198],
                                     sq2[:, ts(c, 512)], start=True,
                                     stop=False)
                    nc.tensor.matmul(psB2[:, ts(c, 512)], k2p_s[:, 198:264],
                                     x1aug[0:64, ts(c, 512)],
                                     start=False, stop=False)
                    nc.tensor.matmul(psB2[:, ts(c, 512)], k2o_s[:, 66:132],
                                     ones_row[:, ts(c, 512)],
                                     start=False, stop=True)
                nc.scalar.activation(B2[:], psB2[:], AF.Copy)
            knn_topk(A2[:], B2[:], 66)
        conv_layer(w4_s[:], bn4s_s[:], bn4b_s[:], 64)

        # ================= SA layers =================
        with tc.tile_pool(name="saw", bufs=1) as saw, \
             tc.tile_pool(name="sal", bufs=1) as sal, \
             tc.tile_pool(name="sas", bufs=2) as sas:
            for l in range(4):
                h = hs[l]
                hn = hs[l + 1]
                qk_s = saw.tile([128, 128], f32, tag="qk")
                nc.sync.dma_start(qk_s[:], qk4[l])
                v_s = saw.tile([128, 128], f32, tag="vs")
                nc.sync.dma_start(v_s[:], v_Tr[l])
                vb_s = saw.tile([128, 128], f32, tag="vb")
                nc.sync.dma_start(vb_s[:], vb_rep[l])
                tT_s = saw.tile([128, 128], f32, tag="tT")
                nc.sync.dma_start(tT_s[:], t_T[l])
                tb_s = saw.tile([128, 1], f32, tag="tb")
                nc.sync.dma_start(tb_s[:], tb_f[l])
                XQ = saw.tile([128, N], f32R, tag="XQ")
                xvT = saw.tile([128, N], f32, tag="xvT")

                with tc.tile_pool(name=f"sap{l}", bufs=1, space="PSUM") as sap:
                    xqp = sap.tile([128, N], f32, tag="xq")
                    for c in range(4):
                        nc.tensor.matmul(xqp[:, ts(c, 512)],
                                         qk_s[:],
                                         h[:, ts(c, 512)],
                                         start=True, stop=True)
                    nc.scalar.activation(XQ[:], xqp[:], AF.Copy)
                with tc.tile_pool(name=f"sav{l}", bufs=1,
                                  space="PSUM") as sav:
                    xvp = sav.tile([128, N], f32, tag="xv")
                    for t in range(NT):
                        nc.tensor.matmul(xvp[:, ts(t, 128)],
                                         h[:, ts(t, 128)],
                                         v_s[:],
                                         start=True, stop=True)
                    nc.vector.tensor_tensor(
                        out=xvT[:].rearrange("p (r c) -> p r c", r=16),
                        in0=xvp[:].rearrange("p (r c) -> p r c", r=16),
                        in1=vb_s[:].rearrange("p (u c) -> p u c", u=1)
                        .broadcast_to((128, 16, 128)),
                        op=OP.add)

                with tc.tile_pool(name=f"sax{l}", bufs=1,
                                  space="PSUM") as sax:
                    xrp = sax.tile([128, N], f32, tag="xr")
                    rcs = sal.tile([64, 1024], f32, tag="rcs")
                    with tc.tile_pool(name=f"sac{l}", bufs=1,
                                      space="PSUM") as sac:
                        csA = sac.tile([64, 512], f32, tag="csA")
                        csB = sac.tile([64, 512], f32, tag="csB")
                        cst4 = [csA[0:1, :], csA[32:33, :],
                                csB[0:1, :], csB[32:33, :]]
                        with tc.tile_pool(name=f"sae{l}", bufs=2,
                                          space="PSUM") as sae:
                            for t in range(NT):
                                ext = sas.tile([128, N], f32R, tag="ex")
                                rs4 = sas.tile([128, 4], f32, tag="rs4")
                                for c in range(4):
                                    q = 32 * (c % 2)
                                    ep = sae.tile([128, 512], f32, tag="ep")
                                    nc.tensor.matmul(
                                        ep[:],
                                        XQ[q:q + 32, ts(t, 128)],
                                        XQ[q:q + 32, ts(c, 512)],
                                        start=True, stop=True)
                                    nc.scalar.activation(
                                        ext[:, ts(c, 512)], ep[:], AF.Exp,
                                        accum_out=rs4[:, c:c + 1])
                                rsum = sas.tile([128, 1], f32, tag="rsum")
                                nc.vector.tensor_reduce(out=rsum[:],
                                                        in_=rs4[:],
                                                        axis=AX.X, op=OP.add)
                                invr = sas.tile([128, 1], f32, tag="invr")
                                nc.vector.reciprocal(invr[:], rsum[:])

                                xvl = sas.tile([128, 128], f32R, tag="xvl")
                                nc.scalar.activation(xvl[:],
                                                     xvT[:, ts(t, 128)],
                                                     AF.Copy, scale=invr[:])
                                for c in range(4):
                                    nc.tensor.matmul(
                                        cst4[c], invr[:],
                                        ext[:, ts(c, 512)].bitcast(f32),
                                        start=(t == 0), stop=(t == NT - 1))
                                for c in range(4):
                                    nc.tensor.matmul(
                                        xrp[:, ts(c, 512)], xvl[:],
                                        ext[:, ts(c, 512)],
                                        start=(t == 0), stop=(t == NT - 1))
                        # 1/(colsum + eps): rows {0,32} x two col halves
                        cssb = sal.tile([64, 1024], f32, tag="cssb")
                        nc.scalar.activation(cssb[:, 0:512], csA[:], AF.Copy)
                        nc.scalar.activation(cssb[:, 512:1024], csB[:],
                                             AF.Copy)
                    # 1/(cs+eps) = exp(-ln(cs+eps)) on ACT (lane-legal)
                    for base in (0, 32):
                        nc.scalar.activation(rcs[base:base + 1, :],
                                             cssb[base:base + 1, :],
                                             AF.Ln)
                        nc.scalar.activation(rcs[base:base + 1, :],
                                             rcs[base:base + 1, :],
                                             AF.Exp, scale=-1.0)
                    rcs4 = [rcs[0:1, 0:512], rcs[32:33, 0:512],
                            rcs[0:1, 512:1024], rcs[32:33, 512:1024]]
                    d = sal.tile([128, N], f32, tag="d")
                    with tc.tile_pool(name=f"sab{l}", bufs=1,
                                      space="PSUM") as sab:
                        rbp = sab.tile([128, N], f32, tag="rb")
                        for c in range(4):
                            base = 32 * (c % 2)
                            nc.tensor.matmul(
                                rbp[:, ts(c, 512)],
                                ones1[base:base + 1, :],
                                rcs4[c],
                                start=True, stop=True)
                        rbs = sal.tile([128, N], f32, tag="rbs")
                        nc.scalar.activation(rbs[:], rbp[:], AF.Copy)
                        xrn = sal.tile([128, N], f32, tag="xrn")
                        nc.vector.tensor_tensor(out=xrn[:], in0=xrp[:],
                                                in1=rbs[:], op=OP.mult)
                        nc.vector.tensor_tensor(out=d[:], in0=h[:],
                                                in1=xrn[:], op=OP.subtract)
                with tc.tile_pool(name=f"sad{l}", bufs=2, space="PSUM") as sad:
                    for c in range(4):
                        drp = sad.tile([128, 512], f32, tag="dr")
                        nc.tensor.matmul(drp[:], tT_s[:],
                                         d[:, ts(c, 512)],
                                         start=True, stop=True)
                        rl = sas.tile([128, 512], f32, tag="rl")
                        nc.scalar.activation(rl[:], drp[:], AF.Relu,
                                             bias=tb_s[:], scale=1.0)
                        nc.vector.tensor_tensor(out=hn[:, ts(c, 512)],
                                                in0=h[:, ts(c, 512)],
                                                in1=rl[:], op=OP.add)

        # ================= fuse =================
        with tc.tile_pool(name="fup", bufs=2, space="PSUM") as fup, \
             tc.tile_pool(name="fus", bufs=2) as fus:
            for o in range(2):
                for c in range(4):
                    fp = fup.tile([128, 512], f32, tag="fp")
                    for k in range(4):
                        nc.tensor.matmul(
                            fp[:],
                            fuse_s[:, 256 * k + 128 * o:
                                   256 * k + 128 * o + 128],
                            hs[k + 1][:, ts(c, 512)],
                            start=(k == 0), stop=(k == 3))
                    yt = fus.tile([128, 512], f32, tag="yt")
                    nc.scalar.activation(yt[:], fp[:], AF.Prelu,
                                         bias=fuse_b_s[:, o:o + 1],
                                         scale=1.0, alpha=0.2)
                    nc.sync.dma_start(y_out[128 * o:128 * o + 128, ts(c, 512)],
                                      yt[:])

    nc.compile()
    return nc


def _prep_inputs(inputs):
    """Host-side: fold BN into weights, build per-core in_maps."""
    f = lambda a: np.ascontiguousarray(np.asarray(a, np.float32))
    x = f(inputs["x"])                      # (8, 2048, 3)
    scale = lambda g: (np.asarray(g, np.float32)
                       / np.sqrt(np.float32(1.0 + EPS_BN)))

    s1, b1 = scale(inputs["g1"]), f(inputs["b1"])
    w1 = f(inputs["w1"])
    w1a = s1[:, None] * w1[:, 0:3]                       # (64, 3)
    w1q = np.concatenate([b1[:, None],
                          s1[:, None] * (w1[:, 3:6] - w1[:, 0:3])],
                         axis=1)                         # (64, 4)
    w1a4 = np.zeros((4, 64), np.float32)
    w1a4[1:4, :] = w1a.T
    s2v, b2v = scale(inputs["g2"]), f(inputs["b2"])
    s3, b3 = scale(inputs["g3"]), f(inputs["b3"])
    w3 = f(inputs["w3"])
    w3a = np.concatenate([s3[:, None] * w3[:, 0:64],
                          np.zeros((64, 1), np.float32)], axis=1)  # (64, 65)
    w3q = np.concatenate([s3[:, None] * (w3[:, 64:128] - w3[:, 0:64]),
                          b3[:, None]], axis=1)          # (64, 65)
    s4v, b4v = scale(inputs["g4"]), f(inputs["b4"])

    qk = f(inputs["sa_qk"])                 # (4, 32, 128)
    v = f(inputs["sa_v"])                   # (4, 128, 128)
    vb = f(inputs["sa_vb"])                 # (4, 128)
    t = f(inputs["sa_t"])
    tb = f(inputs["sa_tb"])
    sg = scale(inputs["sa_g"])              # (4, 128)
    sb = f(inputs["sa_b"])
    fuse_w = f(inputs["fuse_w"])
    sf = scale(inputs["fuse_g"])
    fb = f(inputs["fuse_b"])                # (256,)

    qk_T = np.transpose(qk, (0, 2, 1))      # (4, 128, 32)

    # kNN A/B construction patterns:
    # A1 = [1s; 2x; -xx], B1 = [-xx; x; 1s] (contraction rows 0..4)
    k1As = np.zeros((4, 5), np.float32); k1As[:, 4] = -1.0
    k1Ax = np.zeros((4, 5), np.float32)
    for j in range(3): k1Ax[1 + j, 1 + j] = 2.0
    k1Bs = np.zeros((4, 5), np.float32); k1Bs[:, 0] = -1.0
    k1Bx = np.zeros((4, 5), np.float32)
    for j in range(3): k1Bx[1 + j, 1 + j] = 1.0
    k1o_h = np.zeros((1, 10), np.float32); k1o_h[0, 0] = 1.0; k1o_h[0, 9] = 1.0
    k2As = np.zeros((64, 66), np.float32); k2As[:, 65] = -1.0
    k2Ax = np.zeros((64, 66), np.float32); k2Ax[:, 0:64] = 2.0 * np.eye(64)
    k2Bs = np.zeros((64, 66), np.float32); k2Bs[:, 64] = -1.0
    k2Bx = np.zeros((64, 66), np.float32); k2Bx[:, 0:64] = np.eye(64)
    k2o_h = np.zeros((1, 132), np.float32)
    k2o_h[0, 64] = 1.0; k2o_h[0, 66 + 65] = 1.0

    common = {
        "k1pat": np.ascontiguousarray(
            np.concatenate([k1As, k1Ax, k1Bs, k1Bx], axis=1)),
        "k1o": k1o_h,
        "k2pat": np.ascontiguousarray(
            np.concatenate([k2As, k2Ax, k2Bs, k2Bx], axis=1)),
        "k2o": k2o_h,
        "colenc": np.tile(np.arange(64, dtype=np.uint32), N // 64)[None, :]
                    .repeat(128, 0).copy(),
        "cst": np.array([0xFFFFFFC0, 63, 0xFFF8, 8, 0, 0, 0, 0],
                        dtype=np.uint32)[None, :].repeat(128, 0).copy(),
        "ident64": np.eye(64, dtype=np.float32),
        "w1a_T": np.ascontiguousarray(w1a4),
        "w1q_T": np.ascontiguousarray(w1q.T),
        "w2_Th": np.ascontiguousarray(f(inputs["w2"]).T.astype(np.float16)),
        "bn2_s": np.tile(s2v, 2)[:, None].copy(),
        "bn2_b": np.tile(b2v, 2)[:, None].copy(),
        "w3a_T": np.ascontiguousarray(w3a.T),
        "w3q_T": np.ascontiguousarray(w3q.T),
        "w4_Th": np.ascontiguousarray(f(inputs["w4"]).T.astype(np.float16)),
        "bn4_s": np.tile(s4v, 2)[:, None].copy(),
        "bn4_b": np.tile(b4v, 2)[:, None].copy(),
        "qk4": np.ascontiguousarray(np.tile(qk_T, (1, 1, 4))),
        "v_Tr": np.ascontiguousarray(np.transpose(v, (0, 2, 1))),
        "vb_rep": np.ascontiguousarray(vb[:, None, :].repeat(128, 1)),
        "t_T": np.ascontiguousarray(
            np.transpose(sg[:, :, None] * t, (0, 2, 1))),
        "tb_f": np.ascontiguousarray((sg * tb + sb)[:, :, None]),
        "fuse_T": np.ascontiguousarray((sf[:, None] * fuse_w).T),
        "fuse_bc": np.ascontiguousarray(fb.reshape(2, 128).T),
    }
    in_maps = []
    for b in range(B):
        xt = np.zeros((16, N), np.float32)
        xt[1:4, :] = x[b].T
        m = dict(common)
        m["xt16"] = xt
        in_maps.append(m)
    return in_maps


def kernel(**inputs):
    global _COMPILED
    from concourse.bass_utils import run_bass_kernel_spmd
    if _COMPILED is None:
        _COMPILED = _build_program()
    nc = _COMPILED
    in_maps = _prep_inputs(inputs)
    res = run_bass_kernel_spmd(nc, in_maps, list(range(NCORES)))
    out = np.stack([res.results[b]["y"] for b in range(B)], axis=0)
    return out.astype(np.float32)


# revision 27
# speedup vs baseline: 1.1033x; 1.0085x over previous
"""Trainium2 Bass kernel for nn_Encoder_Head_77343771066713 (DGCNN+PCT encoder).

Data-parallel over batch B=8 across 8 NeuronCores (one point cloud per core).
Self-contained: hardcodes all shapes. kernel(**inputs) -> (8, 256, 2048) f32.

v2 design (vs baseline):
  - neighbor-feature gathers use DMA-engine token gathers (dma_gather with
    16-bit transpose from an SBUF token buffer) instead of gpsimd ap_gather
  - conv layers: A-part (neighbor) conv is pre-applied before the gather
    (tokens = Wa@x per point, fp16); Q-part added post-gather; conv2 via
    column-tiled matmuls packs two row tiles into one PSUM tile; the max
    over k runs on raw conv2 PSUM (valid since BN scale > 0) and BN+leaky
    is applied once on the reduced [128, 1024] stack
  - kNN: -xx[n] row folded into the distance matmul (extra contraction row),
    f32r matmuls, 11-bit index encode into score mantissa (chunk=128 max8,
    no max_index), leaner phase 2
  - SA: replicated XQ enables 32-row-tiled energy matmuls, f32r colsum/
    broadcast matmuls, batched tails, grouped reciprocal
"""
import numpy as np

N = 2048
K = 40
B = 8
NCORES = 8
NT = N // 128          # 16 row tiles
EPS_BN = 1e-5

_COMPILED = None


def _build_program():
    import concourse.bass as bass
    import concourse.tile as tile
    from concourse import bacc, mybir

    f32 = mybir.dt.float32
    f32R = mybir.dt.float32r
    f16 = mybir.dt.float16
    u32 = mybir.dt.uint32
    u16 = mybir.dt.uint16
    i16 = mybir.dt.int16
    AF = mybir.ActivationFunctionType
    OP = mybir.AluOpType
    AX = mybir.AxisListType
    ts = bass.ts

    nc = bacc.Bacc("TRN2", target_bir_lowering=False, debug=False)

    def din(name, shape, dt=f32):
        return nc.dram_tensor(name, shape, dt, kind="ExternalInput")

    xt16 = din("xt16", [16, N])
    colenc = din("colenc", [128, N], u32)
    cst = din("cst", [128, 8], u32)
    ident64 = din("ident64", [64, 64])
    w1a_T = din("w1a_T", [4, 64])
    w1q_T = din("w1q_T", [4, 64])
    w2_Th = din("w2_Th", [64, 64], f16)
    bn2_s = din("bn2_s", [128, 1])
    bn2_b = din("bn2_b", [128, 1])
    w3a_T = din("w3a_T", [65, 64])
    w3q_T = din("w3q_T", [65, 64])
    w4_Th = din("w4_Th", [64, 64], f16)
    bn4_s = din("bn4_s", [128, 1])
    bn4_b = din("bn4_b", [128, 1])
    qk4 = din("qk4", [4, 128, 128])
    v_Tr = din("v_Tr", [4, 128, 128])
    vb_rep = din("vb_rep", [4, 128, 128])
    t_T = din("t_T", [4, 128, 128])
    tb_f = din("tb_f", [4, 128, 1])
    k1pat = din("k1pat", [4, 20])
    k1o = din("k1o", [1, 10])
    k2pat = din("k2pat", [64, 264])
    k2o = din("k2o", [1, 132])
    fuse_T = din("fuse_T", [512, 256])
    fuse_bc = din("fuse_bc", [128, 2])
    y_out = nc.dram_tensor("y", [256, N], f32, kind="ExternalOutput")

    with tile.TileContext(nc) as tc:
      with tc.tile_pool(name="consts", bufs=1) as consts, \
           tc.tile_pool(name="big", bufs=1) as big:
        xt_s = consts.tile([16, N], f32)
        nc.sync.dma_start(xt_s[:], xt16[:])
        colenc_s = consts.tile([128, N], u32)
        nc.sync.dma_start(colenc_s[:], colenc[:])
        cst_s = consts.tile([128, 8], u32)
        nc.sync.dma_start(cst_s[:], cst[:])
        MASKC = cst_s[:, 0:1]     # 0xFFFFFFC0
        C63 = cst_s[:, 1:2]       # 63
        CFFF8 = cst_s[:, 2:3]     # 0xFFF8
        C8 = cst_s[:, 3:4]        # 8
        id64 = consts.tile([64, 64], f32)
        nc.sync.dma_start(id64[:], ident64[:])

        w1a_s = consts.tile([4, 64], f32)
        nc.sync.dma_start(w1a_s[:], w1a_T[:])
        w1q_s = consts.tile([4, 64], f32)
        nc.sync.dma_start(w1q_s[:], w1q_T[:])
        w2_s = consts.tile([64, 64], f16)
        nc.sync.dma_start(w2_s[:], w2_Th[:])
        bn2s_s = consts.tile([128, 1], f32)
        nc.sync.dma_start(bn2s_s[:], bn2_s[:])
        bn2b_s = consts.tile([128, 1], f32)
        nc.sync.dma_start(bn2b_s[:], bn2_b[:])
        w3a_s = consts.tile([65, 64], f32)
        nc.sync.dma_start(w3a_s[:], w3a_T[:])
        w3q_s = consts.tile([65, 64], f32)
        nc.sync.dma_start(w3q_s[:], w3q_T[:])
        w4_s = consts.tile([64, 64], f16)
        nc.sync.dma_start(w4_s[:], w4_Th[:])
        bn4s_s = consts.tile([128, 1], f32)
        nc.sync.dma_start(bn4s_s[:], bn4_s[:])
        bn4b_s = consts.tile([128, 1], f32)
        nc.sync.dma_start(bn4b_s[:], bn4_b[:])
        fuse_s = consts.tile([128, 1024], f32)
        for k in range(4):
            nc.sync.dma_start(fuse_s[:, 256 * k:256 * k + 256],
                              fuse_T[128 * k:128 * k + 128, :])
        fuse_b_s = consts.tile([128, 2], f32)
        nc.sync.dma_start(fuse_b_s[:], fuse_bc[:])
        ones4 = consts.tile([4, 1], f32)
        nc.vector.memset(ones4[:], 1.0)
        ones64 = consts.tile([64, 1], f32)
        nc.vector.memset(ones64[:], 1.0)
        ones1 = consts.tile([128, 128], f32)
        nc.vector.memset(ones1[:], 1.0)
        ones_row = consts.tile([1, N], f32)
        nc.vector.memset(ones_row[:], 1.0)
        k1p_s = consts.tile([4, 20], f32)
        nc.sync.dma_start(k1p_s[:], k1pat[:])
        k1o_s = consts.tile([1, 10], f32)
        nc.sync.dma_start(k1o_s[:], k1o[:])
        k2p_s = consts.tile([64, 264], f32)
        nc.sync.dma_start(k2p_s[:], k2pat[:])
        k2o_s = consts.tile([1, 132], f32)
        nc.sync.dma_start(k2o_s[:], k2o[:])

        # ---- long-lived tensors ----
        tokd = big.tile([2048, 128], f16, tag="tokd", space="DRAM")
        tok = big.tile([128, N], f16, tag="tok")        # token buffer (reused)
        QQb = big.tile([64, N], f16, tag="QQ")          # Q-part (reused)
        idxw = big.tile([128, 5 * 1024], i16, tag="idxw")
        idxq = big.tile([16, 5 * 1024], i16, tag="idxq")
        stg_all = big.tile([128, 40 * NT], u16, tag="stg")
        x_stack = big.tile([128, 1024], f32, tag="xst")
        xsf = big.tile([128, 1024], f32, tag="xsf")
        hs = [big.tile([128, N], f32, name=f"h{i}", tag=f"h{i}")
              for i in range(5)]
        h0 = hs[0]

        # ================= token build =================
        # tok[p, 128r : 128r+64] = fp16(PP[:, 128r + p]); rest zero.
        def build_tokens(lhsA, lhsQ, rhs_ap, nK):
            nc.vector.memset(tok[:], 0.0)
            with tc.tile_pool(name="tbp", bufs=1, space="PSUM") as tbp, \
                 tc.tile_pool(name="tbs", bufs=1) as tbs, \
                 tc.tile_pool(name="trp", bufs=2, space="PSUM") as trp:
                pq_ps = tbp.tile([64, N], f32, tag="pq")
                for c in range(4):
                    nc.tensor.matmul(pq_ps[:, ts(c, 512)],
                                     lhsQ,
                                     rhs_ap[:, ts(c, 512)],
                                     start=True, stop=True)
                nc.scalar.activation(QQb[:], pq_ps[:], AF.Copy)
                for c in range(4):
                    nc.tensor.matmul(pq_ps[:, ts(c, 512)],
                                     lhsA,
                                     rhs_ap[:, ts(c, 512)],
                                     start=True, stop=True)
                pp_sb = tbs.tile([64, N], f32, tag="ppsb")
                nc.scalar.activation(pp_sb[:], pq_ps[:], AF.Copy)
                for r in range(NT):
                    tr = trp.tile([128, 64], f32, tag="tr")
                    nc.tensor.transpose(tr[:], pp_sb[:, ts(r, 128)], id64[:])
                    nc.scalar.activation(tok[:, 128 * r:128 * r + 64], tr[:],
                                         AF.Copy)
                for r in range(NT):
                    nc.sync.dma_start(tokd[128 * r:128 * r + 128, :],
                                      tok[:, 128 * r:128 * r + 128])

        # ================= kNN top-40 =================
        # A/B rows give negdist[n,m] = -xx[n] + 2<x_n,x_m> - xx[m] directly.
        def knn_topk(Amat, Bmat, nK):
            with tc.tile_pool(name="spp", bufs=2, space="PSUM") as spp, \
                 tc.tile_pool(name="scs", bufs=2) as scs, \
                 tc.tile_pool(name="sv2", bufs=2) as sv2:
                for t in range(NT):
                    spt = spp.tile([128, N], f32, tag="spt")
                    lhsT = Amat[:, t * 128:(t + 1) * 128]
                    for c in range(4):
                        nc.tensor.matmul(spt[:, ts(c, 512)],
                                         lhsT,
                                         Bmat[:, ts(c, 512)],
                                         start=True, stop=True)
                    senc = scs.tile([128, N], f32, tag="senc")
                    colv = colenc_s[:, 0:128].rearrange(
                        "p (u c) -> p u c", u=1).broadcast_to((128, 16, 128))
                    nc.vector.scalar_tensor_tensor(
                        out=senc[:].bitcast(u32)
                        .rearrange("p (r c) -> p r c", r=16),
                        in0=spt[:].bitcast(u32)
                        .rearrange("p (r c) -> p r c", r=16),
                        scalar=MASKC, in1=colv,
                        op0=OP.bitwise_and, op1=OP.bitwise_or)
                    sv = sv2.tile([128, 256], f32, tag="sv")
                    for ch in range(32):
                        nc.vector.max(sv[:, 8 * ch:8 * ch + 8],
                                      senc[:, 64 * ch:64 * ch + 64])
                    m40 = sv2.tile([128, 40], f32, tag="m40")
                    pos = sv2.tile([128, 40], u32, tag="pos")
                    wk0 = sv2.tile([128, 256], f32, tag="wk0")
                    wk1 = sv2.tile([128, 256], f32, tag="wk1")
                    cur = sv
                    for r in range(5):
                        nc.vector.max(m40[:, 8 * r:8 * r + 8], cur[:])
                        nc.vector.max_index(pos[:, 8 * r:8 * r + 8],
                                            m40[:, 8 * r:8 * r + 8], cur[:])
                        if r < 4:
                            nxt = wk0 if r % 2 == 0 else wk1
                            nc.vector.match_replace(nxt[:],
                                                    m40[:, 8 * r:8 * r + 8],
                                                    cur[:], -3.0e38)
                            cur = nxt
                    loc = sv2.tile([128, 40], u32, tag="loc")
                    nc.vector.tensor_tensor(out=loc[:],
                                            in0=m40[:].bitcast(u32),
                                            in1=C63.broadcast_to((128, 40)),
                                            op=OP.bitwise_and)
                    chb = sv2.tile([128, 40], u32, tag="chb")
                    nc.vector.tensor_tensor(out=chb[:], in0=pos[:],
                                            in1=CFFF8.broadcast_to((128, 40)),
                                            op=OP.bitwise_and)
                    gid = sv2.tile([128, 40], u32, tag="gid")
                    nc.vector.tensor_tensor(out=gid[:], in0=chb[:],
                                            in1=C8.broadcast_to((128, 40)),
                                            op=OP.mult)
                    nc.vector.tensor_tensor(out=gid[:], in0=gid[:],
                                            in1=loc[:], op=OP.add)
                    nc.vector.tensor_copy(stg_all[:, 40 * t:40 * t + 40],
                                          gid[:])
            # stage indices: idxq[plo, 320t+40phi+j] = stg_all[16phi+plo, 40t+j]
            for phi in range(8):
                nc.sync.dma_start(
                    idxq[:].rearrange("p (t c) -> p t c", t=NT)
                        [:, :, 40 * phi:40 * phi + 40].bitcast(i16),
                    stg_all[16 * phi:16 * phi + 16, :]
                        .rearrange("p (t j) -> p t j", t=NT).bitcast(i16))
            for g in range(8):
                nc.sync.dma_start(idxw[16 * g:16 * g + 16, :], idxq[:])

        # ================= conv layer =================
        # per row tile t: gather 5120 tokens (idx i -> point 16*phi+plo,
        # nbr j with i = 640*phi + 16*j + plo), add Q, leaky, conv2 into
        # packed PSUM (two tiles -> 128 rows), max over j on raw PSUM.
        def conv_layer(w2T, bnS, bnB, hrow):
            with tc.tile_pool(name="gp", bufs=3) as gp, \
                 tc.tile_pool(name="lp", bufs=2) as lp, \
                 tc.tile_pool(name="cp", bufs=2, space="PSUM") as cp:
                for u in range(8):
                    lr = []
                    for half in range(2):
                        t = 2 * u + half
                        g = gp.tile([128, 5120], f16, tag="g")
                        for gc in range(8):
                            nc.gpsimd.dma_gather(
                                out_ap=g[:, 640 * gc:640 * gc + 640]
                                    .rearrange("p (u c) -> p u c", u=1),
                                in_ap=tokd[:],
                                idxs_ap=idxw[:, 320 * t + 40 * gc:
                                             320 * t + 40 * gc + 40],
                                num_idxs=640,
                                num_idxs_reg=640,
                                elem_size=128,
                                transpose=True,
                                queue_num=0)
                        summ = lp.tile([64, 5120], f16, tag="summ")
                        gv = g[0:64, :].rearrange(
                            "c (phi j plo) -> c phi j plo", phi=8, j=40)
                        qv = QQb[:, ts(t, 128)].rearrange(
                            "c (phi uu plo) -> c phi uu plo", uu=1, plo=16) \
                            .broadcast_to((64, 8, 40, 16))
                        sview = summ[:].rearrange(
                            "c (phi j plo) -> c phi j plo", phi=8, j=40)
                        nc.vector.tensor_tensor(out=sview, in0=gv, in1=qv,
                                                op=OP.add)
                        lr1 = lp.tile([64, 5120], f16, tag="lr1")
                        if half == 0:
                            nc.scalar.activation(lr1[:], summ[:], AF.Prelu,
                                                 alpha=0.2)
                        else:
                            nc.vector.scalar_tensor_tensor(
                                out=lr1[:], in0=summ[:], scalar=0.2,
                                in1=summ[:], op0=OP.mult, op1=OP.max)
                        lr.append(lr1)
                    for m in range(4):
                        ps = cp.tile([128, 1536], f32, tag="ps")
                        for half in range(2):
                            rw = 64 * half
                            for (o, w) in ((0, 512), (512, 512), (1024, 256)):
                                nc.tensor.matmul(
                                    ps[rw:rw + 64, o:o + w],
                                    w2T[:],
                                    lr[half][:, 1280 * m + o:1280 * m + o + w],
                                    start=True, stop=True)
                        nc.vector.tensor_reduce(
                            out=x_stack[:, 128 * u + 32 * m:
                                        128 * u + 32 * m + 32],
                            in_=ps[:, 0:1280].rearrange(
                                "c (phi j plo) -> c phi plo j", phi=2, j=40),
                            axis=AX.X, op=OP.max)
            # x = lrelu(bn_s * max + bn_b); write into h0 rows [hrow, hrow+64)
            nc.scalar.activation(xsf[:], x_stack[:], AF.Prelu,
                                 bias=bnB, scale=bnS, alpha=0.2)
            for u in range(8):
                nc.sync.dma_start(h0[hrow:hrow + 64, 256 * u:256 * u + 128],
                                  xsf[0:64, 128 * u:128 * u + 128])
                nc.sync.dma_start(h0[hrow:hrow + 64,
                                     256 * u + 128:256 * u + 256],
                                  xsf[64:128, 128 * u:128 * u + 128])

        # ================= layer 1 =================
        aug4 = big.tile([4, N], f32, tag="aug4")
        nc.scalar.activation(aug4[:], xt_s[0:4, :], AF.Copy)
        nc.vector.memset(aug4[0:1, :], 1.0)
        build_tokens(w1a_s[:], w1q_s[:], aug4[:], 4)

        with tc.tile_pool(name="kn1", bufs=1) as kn1:
            A1 = kn1.tile([5, N], f32, tag="A1")
            B1 = kn1.tile([5, N], f32, tag="B1")
            sq = kn1.tile([4, N], f32, tag="sq1")
            nc.scalar.activation(sq[:], xt_s[0:4, :], AF.Square)
            with tc.tile_pool(name="xxp", bufs=1, space="PSUM") as xxp:
                psA = xxp.tile([5, N], f32, tag="psA")
                psB = xxp.tile([5, N], f32, tag="psB")
                for c in range(4):
                    nc.tensor.matmul(psA[:, ts(c, 512)], k1p_s[:, 0:5],
                                     sq[:, ts(c, 512)], start=True, stop=False)
                    nc.tensor.matmul(psA[:, ts(c, 512)], k1p_s[:, 5:10],
                                     xt_s[0:4, ts(c, 512)],
                                     start=False, stop=False)
                    nc.tensor.matmul(psA[:, ts(c, 512)], k1o_s[:, 0:5],
                                     ones_row[:, ts(c, 512)],
                                     start=False, stop=True)
                    nc.tensor.matmul(psB[:, ts(c, 512)], k1p_s[:, 10:15],
                                     sq[:, ts(c, 512)], start=True, stop=False)
                    nc.tensor.matmul(psB[:, ts(c, 512)], k1p_s[:, 15:20],
                                     xt_s[0:4, ts(c, 512)],
                                     start=False, stop=False)
                    nc.tensor.matmul(psB[:, ts(c, 512)], k1o_s[:, 5:10],
                                     ones_row[:, ts(c, 512)],
                                     start=False, stop=True)
                nc.scalar.activation(A1[:], psA[:], AF.Copy)
                nc.scalar.activation(B1[:], psB[:], AF.Copy)
            knn_topk(A1[:], B1[:], 5)
        conv_layer(w2_s[:], bn2s_s[:], bn2b_s[:], 0)

        # ================= layer 2 =================
        with tc.tile_pool(name="ly2", bufs=1) as ly2:
            x1aug = ly2.tile([66, N], f32, tag="x1aug")
            nc.sync.dma_start(x1aug[0:64, :], h0[0:64, :])
            nc.vector.memset(x1aug[64:65, :], 1.0)
            build_tokens(w3a_s[:], w3q_s[:], x1aug[0:65, :], 65)

            A2 = ly2.tile([66, N], f32, tag="A2")
            B2 = ly2.tile([66, N], f32, tag="B2")
            sq2 = ly2.tile([64, N], f32, tag="sq2")
            nc.scalar.activation(sq2[:], x1aug[0:64, :], AF.Square)
            with tc.tile_pool(name="xx2p", bufs=1, space="PSUM") as xx2p:
                psA2 = xx2p.tile([66, N], f32, tag="psA2")
                for c in range(4):
                    nc.tensor.matmul(psA2[:, ts(c, 512)], k2p_s[:, 0:66],
                                     sq2[:, ts(c, 512)], start=True,
                                     stop=False)
                    nc.tensor.matmul(psA2[:, ts(c, 512)], k2p_s[:, 66:132],
                                     x1aug[0:64, ts(c, 512)],
                                     start=False, stop=False)
                    nc.tensor.matmul(psA2[:, ts(c, 512)], k2o_s[:, 0:66],
                                     ones_row[:, ts(c, 512)],
                                     start=False, stop=True)
                nc.scalar.activation(A2[:], psA2[:], AF.Copy)
            with tc.tile_pool(name="xx3p", bufs=1, space="PSUM") as xx3p:
                psB2 = xx3p.tile([66, N], f32, tag="psB2")
                for c in range(4):
                    nc.tensor.matmul(psB2[:, ts(c, 512)], k2p_s[:, 132:198],
                                     sq2[:, ts(c, 512)], start=True,
                                     stop=False)
                    nc.tensor.matmul(psB2[:, ts(c, 512)], k2p_s[:, 198:264],
                                     x1aug[0:64, ts(c, 512)],
                                     start=False, stop=False)
                    nc.tensor.matmul(psB2[:, ts(c, 512)], k2o_s[:, 66:132],
                                     ones_row[:, ts(c, 512)],
                                     start=False, stop=True)
                nc.scalar.activation(B2[:], psB2[:], AF.Copy)
            knn_topk(A2[:], B2[:], 66)
        conv_layer(w4_s[:], bn4s_s[:], bn4b_s[:], 64)

        # ================= SA layers =================
        with tc.tile_pool(name="saw", bufs=1) as saw, \
             tc.tile_pool(name="sal", bufs=1) as sal, \
             tc.tile_pool(name="sas", bufs=2) as sas:
            for l in range(4):
                h = hs[l]
                hn = hs[l + 1]
                qk_s = saw.tile([128, 128], f32, tag="qk")
                nc.sync.dma_start(qk_s[:], qk4[l])
                v_s = saw.tile([128, 128], f32, tag="vs")
                nc.sync.dma_start(v_s[:], v_Tr[l])
                vb_s = saw.tile([128, 128], f32, tag="vb")
                nc.sync.dma_start(vb_s[:], vb_rep[l])
                tT_s = saw.tile([128, 128], f32, tag="tT")
                nc.sync.dma_start(tT_s[:], t_T[l])
                tb_s = saw.tile([128, 1], f32, tag="tb")
                nc.sync.dma_start(tb_s[:], tb_f[l])
                XQ = saw.tile([128, N], f32R, tag="XQ")
                xvT = saw.tile([128, N], f32, tag="xvT")

                with tc.tile_pool(name=f"sap{l}", bufs=1, space="PSUM") as sap:
                    xqp = sap.tile([128, N], f32, tag="xq")
                    for c in range(4):
                        nc.tensor.matmul(xqp[:, ts(c, 512)],
                                         qk_s[:],
                                         h[:, ts(c, 512)],
                                         start=True, stop=True)
                    nc.scalar.activation(XQ[:], xqp[:], AF.Copy)
                with tc.tile_pool(name=f"sav{l}", bufs=1,
                                  space="PSUM") as sav:
                    xvp = sav.tile([128, N], f32, tag="xv")
                    for t in range(NT):
                        nc.tensor.matmul(xvp[:, ts(t, 128)],
                                         h[:, ts(t, 128)],
                                         v_s[:],
                                         start=True, stop=True)
                    nc.vector.tensor_tensor(
                        out=xvT[:].rearrange("p (r c) -> p r c", r=16),
                        in0=xvp[:].rearrange("p (r c) -> p r c", r=16),
                        in1=vb_s[:].rearrange("p (u c) -> p u c", u=1)
                        .broadcast_to((128, 16, 128)),
                        op=OP.add)

                with tc.tile_pool(name=f"sax{l}", bufs=1,
                                  space="PSUM") as sax:
                    xrp = sax.tile([128, N], f32, tag="xr")
                    rcs = sal.tile([64, 1024], f32, tag="rcs")
                    with tc.tile_pool(name=f"sac{l}", bufs=1,
                                      space="PSUM") as sac:
                        csA = sac.tile([64, 512], f32, tag="csA")
                        csB = sac.tile([64, 512], f32, tag="csB")
                        cst4 = [csA[0:1, :], csA[32:33, :],
                                csB[0:1, :], csB[32:33, :]]
                        with tc.tile_pool(name=f"sae{l}", bufs=2,
                                          space="PSUM") as sae:
                            for t in range(NT):
                                ext = sas.tile([128, N], f32R, tag="ex")
                                rs4 = sas.tile([128, 4], f32, tag="rs4")
                                for c in range(4):
                                    q = 32 * (c % 2)
                                    ep = sae.tile([128, 512], f32, tag="ep")
                                    nc.tensor.matmul(
                                        ep[:],
                                        XQ[q:q + 32, ts(t, 128)],
                                        XQ[q:q + 32, ts(c, 512)],
                                        start=True, stop=True)
                                    nc.scalar.activation(
                                        ext[:, ts(c, 512)], ep[:], AF.Exp,
                                        accum_out=rs4[:, c:c + 1])
                                rsum = sas.tile([128, 1], f32, tag="rsum")
                                nc.vector.tensor_reduce(out=rsum[:],
                                                        in_=rs4[:],
                                                        axis=AX.X, op=OP.add)
                                invr = sas.tile([128, 1], f32, tag="invr")
                                nc.vector.reciprocal(invr[:], rsum[:])

                                xvl = sas.tile([128, 128], f32R, tag="xvl")
                                nc.scalar.activation(xvl[:],
                                                     xvT[:, ts(t, 128)],
                                                     AF.Copy, scale=invr[:])
                                for c in range(4):
                                    nc.tensor.matmul(
                                        cst4[c], invr[:],
                                        ext[:, ts(c, 512)].bitcast(f32),
                                        start=(t == 0), stop=(t == NT - 1))
                                for c in range(4):
                                    nc.tensor.matmul(
                                        xrp[:, ts(c, 512)], xvl[:],
                                        ext[:, ts(c, 512)],
                                        start=(t == 0), stop=(t == NT - 1))
                        # 1/(colsum + eps): rows {0,32} x two col halves
                        cssb = sal.tile([64, 1024], f32, tag="cssb")
                        nc.scalar.activation(cssb[:, 0:512], csA[:], AF.Copy)
                        nc.scalar.activation(cssb[:, 512:1024], csB[:],
                                             AF.Copy)
                    # 1/(cs+eps) = exp(-ln(cs+eps)) on ACT (lane-legal)
                    for base in (0, 32):
                        nc.scalar.activation(rcs[base:base + 1, :],
                                             cssb[base:base + 1, :],
                                             AF.Ln)
                        nc.scalar.activation(rcs[base:base + 1, :],
                                             rcs[base:base + 1, :],
                                             AF.Exp, scale=-1.0)
                    rcs4 = [rcs[0:1, 0:512], rcs[32:33, 0:512],
                            rcs[0:1, 512:1024], rcs[32:33, 512:1024]]
                    d = sal.tile([128, N], f32, tag="d")
                    with tc.tile_pool(name=f"sab{l}", bufs=1,
                                      space="PSUM") as sab:
                        rbp = sab.tile([128, N], f32, tag="rb")
                        for c in range(4):
                            base = 32 * (c % 2)
                            nc.tensor.matmul(
                                rbp[:, ts(c, 512)],
                                ones1[base:base + 1, :],
                                rcs4[c],
                                start=True, stop=True)
                        rbs = sal.tile([128, N], f32, tag="rbs")
                        nc.scalar.activation(rbs[:], rbp[:], AF.Copy)
                        xrn = sal.tile([128, N], f32, tag="xrn")
                        nc.vector.tensor_tensor(out=xrn[:], in0=xrp[:],
                                                in1=rbs[:], op=OP.mult)
                        nc.vector.tensor_tensor(out=d[:], in0=h[:],
                                                in1=xrn[:], op=OP.subtract)
                with tc.tile_pool(name=f"sad{l}", bufs=2, space="PSUM") as sad:
                    for c in range(4):
                        drp = sad.tile([128, 512], f32, tag="dr")
                        nc.tensor.matmul(drp[:], tT_s[:],
                                         d[:, ts(c, 512)],
                                         start=True, stop=True)
                        rl = sas.tile([128, 512], f32, tag="rl")
                        nc.scalar.activation(rl[:], drp[:], AF.Relu,
                                             bias=tb_s[:], scale=1.0)
                        nc.vector.tensor_tensor(out=hn[:, ts(c, 512)],
                                                in0=h[:, ts(c, 512)],
                                                in1=rl[:], op=OP.add)

        # ================= fuse =================
        with tc.tile_pool(name="fup", bufs=2, space="PSUM") as fup, \
             tc.tile_pool(name="fus", bufs=2) as fus:
            for o in range(2):
                for c in range(4):
                    fp = fup.tile([128, 512], f32, tag="fp")
                    for k in range(4):
                        nc.tensor.matmul(
                            fp[:],
                            fuse_s[:, 256 * k + 128 * o:
                                   256 * k + 128 * o + 128],
                            hs[k + 1][:, ts(c, 512)],
                            start=(k == 0), stop=(k == 3))
                    yt = fus.tile([128, 512], f32, tag="yt")
                    nc.scalar.activation(yt[:], fp[:], AF.Prelu,
                                         bias=fuse_b_s[:, o:o + 1],
                                         scale=1.0, alpha=0.2)
                    nc.sync.dma_start(y_out[128 * o:128 * o + 128, ts(c, 512)],
                                      yt[:])

    nc.compile()
    return nc


def _prep_inputs(inputs):
    """Host-side: fold BN into weights, build per-core in_maps."""
    f = lambda a: np.ascontiguousarray(np.asarray(a, np.float32))
    x = f(inputs["x"])                      # (8, 2048, 3)
    scale = lambda g: (np.asarray(g, np.float32)
                       / np.sqrt(np.float32(1.0 + EPS_BN)))

    s1, b1 = scale(inputs["g1"]), f(inputs["b1"])
    w1 = f(inputs["w1"])
    w1a = s1[:, None] * w1[:, 0:3]                       # (64, 3)
    w1q = np.concatenate([b1[:, None],
                          s1[:, None] * (w1[:, 3:6] - w1[:, 0:3])],
                         axis=1)                         # (64, 4)
    w1a4 = np.zeros((4, 64), np.float32)
    w1a4[1:4, :] = w1a.T
    s2v, b2v = scale(inputs["g2"]), f(inputs["b2"])
    s3, b3 = scale(inputs["g3"]), f(inputs["b3"])
    w3 = f(inputs["w3"])
    w3a = np.concatenate([s3[:, None] * w3[:, 0:64],
                          np.zeros((64, 1), np.float32)], axis=1)  # (64, 65)
    w3q = np.concatenate([s3[:, None] * (w3[:, 64:128] - w3[:, 0:64]),
                          b3[:, None]], axis=1)          # (64, 65)
    s4v, b4v = scale(inputs["g4"]), f(inputs["b4"])

    qk = f(inputs["sa_qk"])                 # (4, 32, 128)
    v = f(inputs["sa_v"])                   # (4, 128, 128)
    vb = f(inputs["sa_vb"])                 # (4, 128)
    t = f(inputs["sa_t"])
    tb = f(inputs["sa_tb"])
    sg = scale(inputs["sa_g"])              # (4, 128)
    sb = f(inputs["sa_b"])
    fuse_w = f(inputs["fuse_w"])
    sf = scale(inputs["fuse_g"])
    fb = f(inputs["fuse_b"])                # (256,)

    qk_T = np.transpose(qk, (0, 2, 1))      # (4, 128, 32)

    # kNN A/B construction patterns:
    # A1 = [1s; 2x; -xx], B1 = [-xx; x; 1s] (contraction rows 0..4)
    k1As = np.zeros((4, 5), np.float32); k1As[:, 4] = -1.0
    k1Ax = np.zeros((4, 5), np.float32)
    for j in range(3): k1Ax[1 + j, 1 + j] = 2.0
    k1Bs = np.zeros((4, 5), np.float32); k1Bs[:, 0] = -1.0
    k1Bx = np.zeros((4, 5), np.float32)
    for j in range(3): k1Bx[1 + j, 1 + j] = 1.0
    k1o_h = np.zeros((1, 10), np.float32); k1o_h[0, 0] = 1.0; k1o_h[0, 9] = 1.0
    k2As = np.zeros((64, 66), np.float32); k2As[:, 65] = -1.0
    k2Ax = np.zeros((64, 66), np.float32); k2Ax[:, 0:64] = 2.0 * np.eye(64)
    k2Bs = np.zeros((64, 66), np.float32); k2Bs[:, 64] = -1.0
    k2Bx = np.zeros((64, 66), np.float32); k2Bx[:, 0:64] = np.eye(64)
    k2o_h = np.zeros((1, 132), np.float32)
    k2o_h[0, 64] = 1.0; k2o_h[0, 66 + 65] = 1.0

    common = {
        "k1pat": np.ascontiguousarray(
            np.concatenate([k1As, k1Ax, k1Bs, k1Bx], axis=1)),
        "k1o": k1o_h,
        "k2pat": np.ascontiguousarray(
            np.concatenate([k2As, k2Ax, k2Bs, k2Bx], axis=1)),
        "k2o": k2o_h,
        "colenc": np.tile(np.arange(64, dtype=np.uint32), N // 64)[None, :]
                    .repeat(128, 0).copy(),
        "cst": np.array([0xFFFFFFC0, 63, 0xFFF8, 8, 0, 0, 0, 0],
                        dtype=np.uint32)[None, :].repeat(128, 0).copy(),
        "ident64": np.eye(64, dtype=np.float32),
        "w1a_T": np.ascontiguousarray(w1a4),
        "w1q_T": np.ascontiguousarray(w1q.T),
        "w2_Th": np.ascontiguousarray(f(inputs["w2"]).T.astype(np.float16)),
        "bn2_s": np.tile(s2v, 2)[:, None].copy(),
        "bn2_b": np.tile(b2v, 2)[:, None].copy(),
        "w3a_T": np.ascontiguousarray(w3a.T),
        "w3q_T": np.ascontiguousarray(w3q.T),
        "w4_Th": np.ascontiguousarray(f(inputs["w4"]).T.astype(np.float16)),
        "bn4_s": np.tile(s4v, 2)[:, None].copy(),
        "bn4_b": np.tile(b4v, 2)[:, None].copy(),
        "qk4": np.ascontiguousarray(np.tile(qk_T, (1, 1, 4))),
        "v_Tr": np.ascontiguousarray(np.transpose(v, (0, 2, 1))),
        "vb_rep": np.ascontiguousarray(vb[:, None, :].repeat(128, 1)),
        "t_T": np.ascontiguousarray(
            np.transpose(sg[:, :, None] * t, (0, 2, 1))),
        "tb_f": np.ascontiguousarray((sg * tb + sb)[:, :, None]),
        "fuse_T": np.ascontiguousarray((sf[:, None] * fuse_w).T),
        "fuse_bc": np.ascontiguousarray(fb.reshape(2, 128).T),
    }
    in_maps = []
    for b in range(B):
        xt = np.zeros((16, N), np.float32)
        xt[1:4, :] = x[b].T
        m = dict(common)
        m["xt16"] = xt
        in_maps.append(m)
    return in_maps


def kernel(**inputs):
    global _COMPILED
    from concourse.bass_utils import run_bass_kernel_spmd
    if _COMPILED is None:
        _COMPILED = _build_program()
    nc = _COMPILED
    in_maps = _prep_inputs(inputs)
    res = run_bass_kernel_spmd(nc, in_maps, list(range(NCORES)))
    out = np.stack([res.results[b]["y"] for b in range(B)], axis=0)
    return out.astype(np.float32)
